# revision 21
# baseline (speedup 1.0000x reference)
"""Trainium2 (8 NeuronCores) Bass kernel for the GNN message-passing module.

Math (reference):
  mps  = E_fp[fingerprints]                       [N, d]
  mps  = l2norm_rows(mps + A @ relu(mps @ W_fp + b_fp))
  mpo  = l2norm_axis1(E_bond[bond_index])         [N, N, d] (norm over 2nd atom axis)
  tmp[c,b] = sum_a mps[a,c] * mpo[a,b,c]
  tn[m,d]  = sum_n tmp[d, m*16+n] * mps[m*16+n, d]
  out  = relu(relu(tn@W0+b0)@W1+b1) @ Wp + bp     [B, 1]

Sharding: atom axis a (1024) split 128/core across 8 cores.  Each core
gathers its 128 rows of mpo (bf16, transposed layout [d, a*b]), computes
ssq over b (DVE tensor_tensor_reduce + ACT Square/accum split), and
accumulates tmp[c,b] += diag(w_a) @ slab_a on the TensorEngine.  Per-core
partial tmp is transposed to b-major and ReduceScattered so core k
receives exactly the summed tmp columns of its own 8 molecules; the tiny
MLP finale runs per-core and outputs [8, 1], concatenated on the host.
"""

import sys

for _p in ("/opt/trn_rl_repo",):
    if _p not in sys.path:
        sys.path.insert(0, _p)

import numpy as np
import ml_dtypes

# Some images lack antenv.axon_hooks; bass_utils imports it unguarded when
# trace=True. Provide a shim so the import never crashes (hook stays None ->
# tracing is skipped gracefully unless a real hook is registered).
try:
    import antenv.axon_hooks  # noqa: F401
except ImportError:
    import types as _types

    import antenv as _antenv

    _m = _types.ModuleType("antenv.axon_hooks")
    _m._hook = None

    def _set_hook(h):
        _m._hook = h

    def _get_hook():
        return _m._hook

    _m.set_axon_ntff_profile_hook = _set_hook
    _m.get_axon_ntff_profile_hook = _get_hook
    sys.modules["antenv.axon_hooks"] = _m
    _antenv.axon_hooks = _m

import concourse.bacc as bacc
import concourse.mybir as mybir
import concourse.tile as tile
from concourse.bass_utils import run_bass_kernel_spmd

BF16 = ml_dtypes.bfloat16

NCORES = 8
N = 1024          # total atoms
DIM = 128
B = 64            # molecules
NA = 16           # atoms per molecule
APC = N // NCORES  # atoms per core = 128
MPC = B // NCORES  # molecules per core = 8
NCH = 8           # compute groups per core
ACH = APC // NCH  # a-rows per group = 16
GCALLS = 4        # gather calls per group (ring carveout limits a call
                  # to ~1000 descriptors = ~4K indices with pipelining room)
GIDX = ACH * N // GCALLS  # 4096 indices per gather call
N_FP = 10000
N_BOND = 10000
EPS = 1e-12
# Of each group's 16 a-rows, how many take the DVE (square + tree-add)
# ssq path; the rest go to ACT (Square + accum_out).
DVE_J = 7

F32 = mybir.dt.float32
BF = mybir.dt.bfloat16
I16 = mybir.dt.int16
AOT = mybir.AluOpType
AFT = mybir.ActivationFunctionType


def _wrap_idx16(flat):
    """SWDGE dma_gather index layout: idx i -> partition i%16, col i//16,
    replicated across the 8 Q7 cores (rows 16..127)."""
    flat = np.ascontiguousarray(flat.astype(np.int16))
    n = flat.shape[0]
    assert n % 16 == 0
    w = flat.reshape(n // 16, 16).T  # [16, n//16]
    return np.tile(w, (8, 1))        # [128, n//16]


def build_nc(stage="full"):
    nc = bacc.Bacc("TRN2", target_bir_lowering=False, debug=False,
                   num_devices=NCORES)

    e_bond = nc.declare_dram_parameter("e_bond", [N_BOND, DIM], BF, False)
    e_fp = nc.declare_dram_parameter("e_fp", [N_FP, DIM], BF, False)
    fpi_all = nc.declare_dram_parameter("fpi_all", [128, N // 16], I16, False)
    fpi_own = nc.declare_dram_parameter("fpi_own", [128, APC // 16], I16, False)
    bidx = nc.declare_dram_parameter("bidx", [128, APC * N // 16], I16, False)
    at = nc.declare_dram_parameter("at", [N, APC], BF, False)
    wfp = nc.declare_dram_parameter("wfp", [DIM, DIM], BF, False)
    bfp = nc.declare_dram_parameter("bfp", [1, DIM], BF, False)
    w0 = nc.declare_dram_parameter("w0", [DIM, DIM], F32, False)
    b0 = nc.declare_dram_parameter("b0", [1, DIM], F32, False)
    w1 = nc.declare_dram_parameter("w1", [DIM, DIM], F32, False)
    b1 = nc.declare_dram_parameter("b1", [1, DIM], F32, False)
    wp = nc.declare_dram_parameter("wp", [DIM, 1], F32, False)
    bp = nc.declare_dram_parameter("bp", [1, 1], F32, False)
    idbf = nc.declare_dram_parameter("idbf", [128, 128], BF, False)
    idf = nc.declare_dram_parameter("idf", [128, 128], F32, False)
    onesbf = nc.declare_dram_parameter("onesbf", [1, 128], BF, False)
    onesf = nc.declare_dram_parameter("onesf", [1, 128], F32, False)
    sel = nc.declare_dram_parameter("sel", [128, MPC], F32, False)
    out = nc.declare_dram_parameter("out", [MPC, 1], F32, True)

    cc_in = nc.dram_tensor("cc_in", [N, DIM], F32)
    cc_out = nc.dram_tensor("cc_out", [APC, DIM], F32)

    with tile.TileContext(nc) as tc:
        with (
            tc.tile_pool(name="const", bufs=1) as cp,
            tc.tile_pool(name="slab", bufs=2) as slabp,
            tc.tile_pool(name="scr", bufs=2) as scrp,
            tc.tile_pool(name="small", bufs=2) as smp,
            tc.tile_pool(name="diag", bufs=4) as diagp,
            tc.tile_pool(name="psA", bufs=1, space="PSUM") as psA,
            tc.tile_pool(name="psB", bufs=3, space="PSUM") as psB,
        ):
            # ---- constants to SBUF -------------------------------------
            wfp_s = cp.tile([DIM, DIM], BF, tag="wfp_s")
            nc.sync.dma_start(wfp_s[:], wfp[:])
            idbf_s = cp.tile([128, 128], BF, tag="idbf_s")
            nc.sync.dma_start(idbf_s[:], idbf[:])
            idf_s = cp.tile([128, 128], F32, tag="idf_s")
            nc.sync.dma_start(idf_s[:], idf[:])
            onesbf_s = cp.tile([1, 128], BF, tag="onesbf_s")
            nc.sync.dma_start(onesbf_s[:], onesbf[:])
            onesf_s = cp.tile([1, 128], F32, tag="onesf_s")
            nc.sync.dma_start(onesf_s[:], onesf[:])
            bfp_s = cp.tile([1, DIM], BF, tag="bfp_s")
            nc.sync.dma_start(bfp_s[:], bfp[:])
            w0_s = cp.tile([DIM, DIM], F32, tag="w0_s")
            nc.sync.dma_start(w0_s[:], w0[:])
            b0_s = cp.tile([1, DIM], F32, tag="b0_s")
            nc.sync.dma_start(b0_s[:], b0[:])
            w1_s = cp.tile([DIM, DIM], F32, tag="w1_s")
            nc.sync.dma_start(w1_s[:], w1[:])
            b1_s = cp.tile([1, DIM], F32, tag="b1_s")
            nc.sync.dma_start(b1_s[:], b1[:])
            wp_s = cp.tile([DIM, 1], F32, tag="wp_s")
            nc.sync.dma_start(wp_s[:], wp[:])
            bp_s = cp.tile([1, 1], F32, tag="bp_s")
            nc.sync.dma_start(bp_s[:], bp[:])
            sel_s = cp.tile([128, MPC], F32, tag="sel_s")
            nc.sync.dma_start(sel_s[:], sel[:])
            fpi_all_s = cp.tile([128, N // 16], I16, tag="fpi_all_s")
            nc.sync.dma_start(fpi_all_s[:], fpi_all[:])
            fpi_own_s = cp.tile([128, APC // 16], I16, tag="fpi_own_s")
            nc.sync.dma_start(fpi_own_s[:], fpi_own[:])
            bidx_s = cp.tile([128, APC * N // 16], I16, tag="bidx_s")
            nc.sync.dma_start(bidx_s[:], bidx[:])
            at_s = cp.tile([128, NCH, 128], BF, tag="at_s")
            for j in range(NCH):
                nc.sync.dma_start(at_s[:, j, :], at[j * 128:(j + 1) * 128, :])

            # ---- MPS stage ---------------------------------------------
            # mps0T: [c, b] bf16 for all 1024 atoms (replicated compute)
            mps0T = cp.tile([128, 1, N], BF, tag="mps0T")
            nc.gpsimd.dma_gather(
                out_ap=mps0T[:], in_ap=e_fp[:], idxs_ap=fpi_all_s[:],
                num_idxs=N, num_idxs_reg=N, elem_size=DIM, transpose=True,
                single_packet=False)
            # mps0 for own rows, [c, a_own]
            mps0oT = cp.tile([128, 1, APC], BF, tag="mps0oT")
            nc.gpsimd.dma_gather(
                out_ap=mps0oT[:], in_ap=e_fp[:], idxs_ap=fpi_own_s[:],
                num_idxs=APC, num_idxs_reg=APC, elem_size=DIM, transpose=True,
                single_packet=False)

            # contri[b, c'] = relu(mps0 @ W_fp + b_fp), chunked over b
            contri_s = cp.tile([128, NCH, DIM], BF, tag="contri_s")
            for j in range(NCH):
                cps = psB.tile([128, DIM], F32, tag="ps")
                nc.tensor.matmul(cps[:], lhsT=mps0T[:, 0, j * 128:(j + 1) * 128],
                                 rhs=wfp_s[:], start=True, stop=False)
                nc.tensor.matmul(cps[:], lhsT=onesbf_s[:], rhs=bfp_s[:],
                                 start=False, stop=True)
                nc.vector.tensor_scalar_max(contri_s[:, j, :], cps[:], 0.0)

            # mps_own[a, c] = mps0_own + A[own rows] @ contri   (dense)
            mps_ps = psB.tile([128, DIM], F32, tag="ps")
            for j in range(NCH):
                nc.tensor.matmul(mps_ps[:], lhsT=at_s[:, j, :],
                                 rhs=contri_s[:, j, :],
                                 start=(j == 0), stop=False)
            nc.tensor.matmul(mps_ps[:], lhsT=mps0oT[:, 0, :], rhs=idbf_s[:],
                             start=False, stop=True)
            mps_own = cp.tile([128, DIM], F32, tag="mps_own")
            nc.vector.tensor_copy(mps_own[:], mps_ps[:])

            # l2 normalize rows (free axis)
            nsq = smp.tile([128, 1], F32, tag="nsq")
            nscr = smp.tile([128, DIM], F32, tag="nscr")
            nc.scalar.activation(out=nscr[:], in_=mps_own[:],
                                 func=AFT.Square, accum_out=nsq[:])
            nrm = smp.tile([128, 1], F32, tag="nrm")
            nc.scalar.sqrt(nrm[:], nsq[:])
            nrm2 = smp.tile([128, 1], F32, tag="nrm2")
            nc.vector.tensor_scalar_max(nrm2[:], nrm[:], EPS)
            inv = smp.tile([128, 1], F32, tag="inv")
            nc.vector.reciprocal(inv[:], nrm2[:])
            mps_n = cp.tile([128, DIM], F32, tag="mps_n")  # [a_own, c]
            nc.vector.tensor_scalar_mul(mps_n[:], mps_own[:], inv[:])
            # transpose -> [c, a_own]
            mnt_ps = psB.tile([128, 128], F32, tag="ps")
            nc.tensor.transpose(mnt_ps[:], mps_n[:], idf_s[:])
            mps_nT = cp.tile([128, 128], F32, tag="mps_nT")
            nc.vector.tensor_copy(mps_nT[:], mnt_ps[:])

            # ---- main loop: mpo gather + ssq + diag matmuls ------------
            ssq = cp.tile([128, APC], F32, tag="ssq")   # [c, a_local]
            wT = cp.tile([128, APC], F32, tag="wT")     # [c, a_local]
            tmp_ps = psA.tile([128, N], F32, tag="tmp_ps")  # [c, b] accum

            nch_eff = int(stage[1:]) if stage.startswith("g") else NCH
            for ch in range(nch_eff):
                slab = slabp.tile([128, 1, ACH * N], BF, tag="slab")
                # 4 gather calls of 4096 idxs each (SWDGE ring carveout
                # holds ~1000 descriptors; 258/call leaves pipelining room)
                for q in range(GCALLS):
                    i0 = ch * (ACH * N // 16) + q * (GIDX // 16)
                    nc.gpsimd.dma_gather(
                        out_ap=slab[:, :, q * GIDX:(q + 1) * GIDX],
                        in_ap=e_bond[:],
                        idxs_ap=bidx_s[:, i0:i0 + GIDX // 16],
                        num_idxs=GIDX, num_idxs_reg=GIDX,
                        elem_size=DIM, transpose=True, single_packet=False)

                if stage == "gather":
                    gdump = smp.tile([128, GCALLS], BF, tag="gdump")
                    for q in range(GCALLS):
                        nc.vector.tensor_copy(gdump[:, q:q + 1],
                                              slab[:, 0, q * GIDX:q * GIDX + 1])
                    continue
                # ssq: first DVE_J rows on DVE (square + tree-add),
                # the rest on ACT (Square + accum_out).
                if DVE_J > 0:
                    sq = scrp.tile([128, DVE_J, N], BF, tag="dscr")
                    nc.vector.tensor_mul(
                        sq.rearrange("p j n -> p (j n)"),
                        slab[:, 0, :DVE_J * N], slab[:, 0, :DVE_J * N])
                    # tree-add over b within each row
                    t1 = scrp.tile([128, DVE_J, N // 2], BF, tag="tr1")
                    t2 = scrp.tile([128, DVE_J, N // 4], BF, tag="tr2")
                    nc.vector.tensor_add(t1[:], sq[:, :, :N // 2],
                                         sq[:, :, N // 2:])
                    nc.vector.tensor_add(t2[:], t1[:, :, :N // 4],
                                         t1[:, :, N // 4:])
                    lvls = [t2]
                    w_ = N // 4
                    while w_ > 2:
                        w_ //= 2
                        nxt = scrp.tile([128, DVE_J, w_], BF,
                                        tag=f"tr{w_}")
                        nc.vector.tensor_add(nxt[:], lvls[-1][:, :, :w_],
                                             lvls[-1][:, :, w_:])
                        lvls.append(nxt)
                    # final level -> f32 ssq columns
                    last = lvls[-1]
                    nc.vector.tensor_add(
                        ssq[:, ch * ACH:ch * ACH + DVE_J],
                        last[:, :, 0], last[:, :, 1])
                for j in range(DVE_J, ACH):
                    al = ch * ACH + j
                    scr = scrp.tile([128, N], BF, tag="ascr")
                    nc.scalar.activation(
                        out=scr[:], in_=slab[:, 0, j * N:(j + 1) * N],
                        func=AFT.Square, accum_out=ssq[:, al:al + 1])

                if stage == "ssq":
                    continue
                # w for this chunk: w[c, a] = mps_nT / max(sqrt(ssq), eps)
                c0, c1 = ch * ACH, (ch + 1) * ACH
                st = smp.tile([128, ACH], F32, tag="st")
                nc.scalar.sqrt(st[:], ssq[:, c0:c1])
                st2 = smp.tile([128, ACH], F32, tag="st2")
                nc.vector.tensor_scalar_max(st2[:], st[:], EPS)
                sti = smp.tile([128, ACH], F32, tag="sti")
                nc.vector.reciprocal(sti[:], st2[:])
                nc.vector.tensor_mul(wT[:, c0:c1], mps_nT[:, c0:c1], sti[:])

                # tmp[c, b] += diag(w_a) @ slab_a
                for j in range(ACH):
                    al = ch * ACH + j
                    diag = diagp.tile([128, 128], BF, tag="diag")
                    nc.vector.tensor_scalar_mul(diag[:], idbf_s[:],
                                                wT[:, al:al + 1])
                    nc.tensor.matmul(
                        tmp_ps[:, 0:512], lhsT=diag[:],
                        rhs=slab[:, 0, j * N:j * N + 512],
                        start=(al == 0), stop=(al == nch_eff * ACH - 1),
                        skip_group_check=True)
                    nc.tensor.matmul(
                        tmp_ps[:, 512:1024], lhsT=diag[:],
                        rhs=slab[:, 0, j * N + 512:(j + 1) * N],
                        start=(al == 0), stop=(al == nch_eff * ACH - 1),
                        skip_group_check=True)

            if stage in ("gather", "ssq", "mm"):  # early-exit debug stages
                # debug early-exit: emit a token output and stop
                dbg = smp.tile([MPC, 1], F32, tag="dbg")
                if stage == "gather":
                    nc.vector.tensor_copy(dbg[:], slab[:MPC, 0, 0:1])
                elif stage == "ssq":
                    nc.vector.tensor_copy(dbg[:], ssq[:MPC, 0:1])
                else:
                    tmp_dbg = cp.tile([128, N], F32, tag="tmp_dbg")
                    nc.vector.tensor_copy(tmp_dbg[:], tmp_ps[:])
                    nc.vector.tensor_copy(dbg[:], tmp_dbg[:MPC, 0:1])
                nc.sync.dma_start(out[:], dbg[:])
                nc.compile()
                return nc

            # ---- tmp -> b-major -> ReduceScatter -----------------------
            tmp_s = cp.tile([128, N], F32, tag="tmp_s")
            nc.vector.tensor_copy(tmp_s[:], tmp_ps[:])
            for j in range(NCH):
                tps = psB.tile([128, 128], F32, tag="ps")
                nc.tensor.transpose(tps[:], tmp_s[:, j * 128:(j + 1) * 128],
                                    idf_s[:])
                tts = smp.tile([128, 128], F32, tag="tts")
                nc.vector.tensor_copy(tts[:], tps[:])
                nc.sync.dma_start(cc_in[j * 128:(j + 1) * 128, :], tts[:])

            nc.gpsimd.collective_compute(
                "ReduceScatter", AOT.add,
                replica_groups=[list(range(NCORES))],
                ins=[cc_in[:]], outs=[cc_out[:]])

            # ---- finale: tn + MLP (own 8 molecules) --------------------
            tro = smp.tile([128, DIM], F32, tag="tro")  # [b_own, c]
            nc.sync.dma_start(tro[:], cc_out[:])
            prod = smp.tile([128, DIM], F32, tag="prod")
            nc.vector.tensor_mul(prod[:], tro[:], mps_n[:])
            tn_ps = psB.tile([MPC, DIM], F32, tag="ps")
            nc.tensor.matmul(tn_ps[:], lhsT=sel_s[:], rhs=prod[:],
                             start=True, stop=True)
            tn_s = smp.tile([MPC, DIM], F32, tag="tn_s")
            nc.vector.tensor_copy(tn_s[:], tn_ps[:])
            tnT_ps = psB.tile([128, MPC], F32, tag="ps")
            nc.tensor.transpose(tnT_ps[:], tn_s[:], idf_s[:MPC, :MPC])
            tnT_s = smp.tile([128, MPC], F32, tag="tnT_s")
            nc.vector.tensor_copy(tnT_s[:], tnT_ps[:])

            x0_ps = psB.tile([MPC, DIM], F32, tag="ps")
            nc.tensor.matmul(x0_ps[:], lhsT=tnT_s[:], rhs=w0_s[:],
                             start=True, stop=False)
            nc.tensor.matmul(x0_ps[:], lhsT=onesf_s[:, :MPC], rhs=b0_s[:],
                             start=False, stop=True)
            x0_s = smp.tile([MPC, DIM], F32, tag="x0_s")
            nc.vector.tensor_scalar_max(x0_s[:], x0_ps[:], 0.0)
            x0T_ps = psB.tile([128, MPC], F32, tag="ps")
            nc.tensor.transpose(x0T_ps[:], x0_s[:], idf_s[:MPC, :MPC])
            x0T_s = smp.tile([128, MPC], F32, tag="x0T_s")
            nc.vector.tensor_copy(x0T_s[:], x0T_ps[:])

            x1_ps = psB.tile([MPC, DIM], F32, tag="ps")
            nc.tensor.matmul(x1_ps[:], lhsT=x0T_s[:], rhs=w1_s[:],
                             start=True, stop=False)
            nc.tensor.matmul(x1_ps[:], lhsT=onesf_s[:, :MPC], rhs=b1_s[:],
                             start=False, stop=True)
            x1_s = smp.tile([MPC, DIM], F32, tag="x1_s")
            nc.vector.tensor_scalar_max(x1_s[:], x1_ps[:], 0.0)
            x1T_ps = psB.tile([128, MPC], F32, tag="ps")
            nc.tensor.transpose(x1T_ps[:], x1_s[:], idf_s[:MPC, :MPC])
            x1T_s = smp.tile([128, MPC], F32, tag="x1T_s")
            nc.vector.tensor_copy(x1T_s[:], x1T_ps[:])

            y_ps = psB.tile([MPC, 1], F32, tag="ps")
            nc.tensor.matmul(y_ps[:], lhsT=x1T_s[:], rhs=wp_s[:],
                             start=True, stop=False)
            nc.tensor.matmul(y_ps[:], lhsT=onesf_s[:, :MPC], rhs=bp_s[:, :1],
                             start=False, stop=True)
            y_s = smp.tile([MPC, 1], F32, tag="y_s")
            nc.vector.tensor_copy(y_s[:], y_ps[:])
            nc.sync.dma_start(out[:], y_s[:])

    nc.compile()
    return nc


_NC_CACHE = None


def _get_nc():
    global _NC_CACHE
    if _NC_CACHE is None:
        _NC_CACHE = build_nc()
    return _NC_CACHE


def make_in_maps(fingerprints, adjacency, bond_index, E_fp, E_bond, W_fp,
                 b_fp, W_out0, b_out0, W_out1, b_out1, W_prop, b_prop):
    e_bond_bf = np.ascontiguousarray(E_bond.astype(BF16))
    e_fp_bf = np.ascontiguousarray(E_fp.astype(BF16))
    wfp_bf = np.ascontiguousarray(W_fp.astype(BF16))
    bfp_bf = b_fp.astype(BF16).reshape(1, DIM)
    w0_f = np.ascontiguousarray(W_out0.astype(np.float32))
    b0_f = b_out0.astype(np.float32).reshape(1, DIM)
    w1_f = np.ascontiguousarray(W_out1.astype(np.float32))
    b1_f = b_out1.astype(np.float32).reshape(1, DIM)
    wp_f = np.ascontiguousarray(W_prop.astype(np.float32))
    bp_f = b_prop.astype(np.float32).reshape(1, 1)
    idbf = np.eye(128, dtype=BF16)
    idf = np.eye(128, dtype=np.float32)
    onesbf = np.ones((1, 128), dtype=BF16)
    onesf = np.ones((1, 128), dtype=np.float32)
    # molecule-sum selector: sel[b, m] = 1 if b // 16 == m
    sel = np.zeros((128, MPC), dtype=np.float32)
    for m in range(MPC):
        sel[m * NA:(m + 1) * NA, m] = 1.0

    fpi_all = _wrap_idx16(fingerprints)

    in_maps = []
    for k in range(NCORES):
        rows = slice(k * APC, (k + 1) * APC)
        # bond idx: wrapped per gather call (each call wraps its own idxs)
        flat = bond_index[rows, :].astype(np.int16).reshape(
            NCH * GCALLS, GIDX)
        bidx = np.concatenate(
            [_wrap_idx16(flat[c]) for c in range(NCH * GCALLS)],
            axis=1)  # [128, 8192]
        at_k = np.ascontiguousarray(adjacency[rows, :].T.astype(BF16))
        fpi_own = _wrap_idx16(fingerprints[rows])
        in_maps.append({
            "e_bond": e_bond_bf, "e_fp": e_fp_bf,
            "fpi_all": fpi_all, "fpi_own": fpi_own, "bidx": bidx,
            "at": at_k, "wfp": wfp_bf, "bfp": bfp_bf,
            "w0": w0_f, "b0": b0_f, "w1": w1_f, "b1": b1_f,
            "wp": wp_f, "bp": bp_f,
            "idbf": idbf, "idf": idf, "onesbf": onesbf, "onesf": onesf,
            "sel": sel,
        })
    return in_maps


def run(inputs, trace=False, **kw):
    nc = _get_nc()
    in_maps = make_in_maps(**inputs)
    res = run_bass_kernel_spmd(nc, in_maps, core_ids=list(range(NCORES)),
                               trace=trace, **kw)
    out = np.concatenate([res.results[k]["out"] for k in range(NCORES)],
                         axis=0).astype(np.float32)
    return out, res


def kernel(**inputs):
    out, _ = run(inputs, trace=False)
    return out


# revision 23
# speedup vs baseline: 15.2626x; 15.2626x over previous
"""Trainium2 (8 NeuronCores) Bass kernel for the GNN message-passing module.

Math (reference):
  mps  = E_fp[fingerprints]                       [N, d]
  mps  = l2norm_rows(mps + A @ relu(mps @ W_fp + b_fp))
  mpo  = l2norm_axis1(E_bond[bond_index])         [N, N, d] (norm over 2nd atom axis)
  tmp[c,b] = sum_a mps[a,c] * mpo[a,b,c]
  tn[m,d]  = sum_n tmp[d, m*16+n] * mps[m*16+n, d]
  out  = relu(relu(tn@W0+b0)@W1+b1) @ Wp + bp     [B, 1]

Sharding: atom axis a (1024) split 128/core across 8 cores.  Each core
gathers its 128 rows of mpo (bf16, transposed layout [d, a*b]), computes
ssq over b (DVE tensor_tensor_reduce + ACT Square/accum split), and
accumulates tmp[c,b] += diag(w_a) @ slab_a on the TensorEngine.  Per-core
partial tmp is transposed to b-major and ReduceScattered so core k
receives exactly the summed tmp columns of its own 8 molecules; the tiny
MLP finale runs per-core and outputs [8, 1], concatenated on the host.
"""

import sys

for _p in ("/opt/trn_rl_repo",):
    if _p not in sys.path:
        sys.path.insert(0, _p)

import numpy as np
import ml_dtypes

# Some images lack antenv.axon_hooks; bass_utils imports it unguarded when
# trace=True. Provide a shim so the import never crashes (hook stays None ->
# tracing is skipped gracefully unless a real hook is registered).
try:
    import antenv.axon_hooks  # noqa: F401
except ImportError:
    import types as _types

    import antenv as _antenv

    _m = _types.ModuleType("antenv.axon_hooks")
    _m._hook = None

    def _set_hook(h):
        _m._hook = h

    def _get_hook():
        return _m._hook

    _m.set_axon_ntff_profile_hook = _set_hook
    _m.get_axon_ntff_profile_hook = _get_hook
    sys.modules["antenv.axon_hooks"] = _m
    _antenv.axon_hooks = _m

import concourse.bacc as bacc
import concourse.mybir as mybir
import concourse.tile as tile
from concourse.bass_utils import run_bass_kernel_spmd

BF16 = ml_dtypes.bfloat16

NCORES = 8
N = 1024          # total atoms
DIM = 128
B = 64            # molecules
NA = 16           # atoms per molecule
APC = N // NCORES  # atoms per core = 128
MPC = B // NCORES  # molecules per core = 8
NCH = 8           # compute groups per core
ACH = APC // NCH  # a-rows per group = 16
GCALLS = 4        # gather calls per group (ring carveout limits a call
                  # to ~1000 descriptors = ~4K indices with pipelining room)
GIDX = ACH * N // GCALLS  # 4096 indices per gather call
N_FP = 10000
N_BOND = 10000
EPS = 1e-12
# Of each group's 16 a-rows, how many take the DVE (square + tree-add)
# ssq path; the rest go to ACT (Square + accum_out).
DVE_J = 7

F32 = mybir.dt.float32
BF = mybir.dt.bfloat16
I16 = mybir.dt.int16
AOT = mybir.AluOpType
AFT = mybir.ActivationFunctionType


def _wrap_idx16(flat):
    """SWDGE dma_gather index layout: idx i -> partition i%16, col i//16,
    replicated across the 8 Q7 cores (rows 16..127)."""
    flat = np.ascontiguousarray(flat.astype(np.int16))
    n = flat.shape[0]
    assert n % 16 == 0
    w = flat.reshape(n // 16, 16).T  # [16, n//16]
    return np.tile(w, (8, 1))        # [128, n//16]


def build_nc(stage="full"):
    nc = bacc.Bacc("TRN2", target_bir_lowering=False, debug=False,
                   num_devices=NCORES)

    e_bond = nc.declare_dram_parameter("e_bond", [N_BOND, DIM], BF, False)
    e_fp = nc.declare_dram_parameter("e_fp", [N_FP, DIM], BF, False)
    fpi_all = nc.declare_dram_parameter("fpi_all", [128, N // 16], I16, False)
    fpi_own = nc.declare_dram_parameter("fpi_own", [128, APC // 16], I16, False)
    bidx = nc.declare_dram_parameter("bidx", [128, APC * N // 16], I16, False)
    at = nc.declare_dram_parameter("at", [N, APC], BF, False)
    wfp = nc.declare_dram_parameter("wfp", [DIM, DIM], BF, False)
    bfp = nc.declare_dram_parameter("bfp", [1, DIM], BF, False)
    w0 = nc.declare_dram_parameter("w0", [DIM, DIM], F32, False)
    b0 = nc.declare_dram_parameter("b0", [1, DIM], F32, False)
    w1 = nc.declare_dram_parameter("w1", [DIM, DIM], F32, False)
    b1 = nc.declare_dram_parameter("b1", [1, DIM], F32, False)
    wp = nc.declare_dram_parameter("wp", [DIM, 1], F32, False)
    bp = nc.declare_dram_parameter("bp", [1, 1], F32, False)
    idbf = nc.declare_dram_parameter("idbf", [128, 128], BF, False)
    idf = nc.declare_dram_parameter("idf", [128, 128], F32, False)
    onesbf = nc.declare_dram_parameter("onesbf", [1, 128], BF, False)
    onesf = nc.declare_dram_parameter("onesf", [1, 128], F32, False)
    sel = nc.declare_dram_parameter("sel", [128, MPC], F32, False)
    out = nc.declare_dram_parameter("out", [MPC, 1], F32, True)

    cc_in = nc.dram_tensor("cc_in", [N, DIM], F32)
    cc_out = nc.dram_tensor("cc_out", [APC, DIM], F32)

    with tile.TileContext(nc) as tc:
        with (
            tc.tile_pool(name="const", bufs=1) as cp,
            tc.tile_pool(name="slab", bufs=2) as slabp,
            tc.tile_pool(name="scr", bufs=2) as scrp,
            tc.tile_pool(name="small", bufs=2) as smp,
            tc.tile_pool(name="diag", bufs=4) as diagp,
            tc.tile_pool(name="psA", bufs=1, space="PSUM") as psA,
            tc.tile_pool(name="psB", bufs=3, space="PSUM") as psB,
        ):
            # ---- constants to SBUF -------------------------------------
            wfp_s = cp.tile([DIM, DIM], BF, tag="wfp_s")
            nc.sync.dma_start(wfp_s[:], wfp[:])
            idbf_s = cp.tile([128, 128], BF, tag="idbf_s")
            nc.sync.dma_start(idbf_s[:], idbf[:])
            idf_s = cp.tile([128, 128], F32, tag="idf_s")
            nc.sync.dma_start(idf_s[:], idf[:])
            onesbf_s = cp.tile([1, 128], BF, tag="onesbf_s")
            nc.sync.dma_start(onesbf_s[:], onesbf[:])
            onesf_s = cp.tile([1, 128], F32, tag="onesf_s")
            nc.sync.dma_start(onesf_s[:], onesf[:])
            bfp_s = cp.tile([1, DIM], BF, tag="bfp_s")
            nc.sync.dma_start(bfp_s[:], bfp[:])
            w0_s = cp.tile([DIM, DIM], F32, tag="w0_s")
            nc.sync.dma_start(w0_s[:], w0[:])
            b0_s = cp.tile([1, DIM], F32, tag="b0_s")
            nc.sync.dma_start(b0_s[:], b0[:])
            w1_s = cp.tile([DIM, DIM], F32, tag="w1_s")
            nc.sync.dma_start(w1_s[:], w1[:])
            b1_s = cp.tile([1, DIM], F32, tag="b1_s")
            nc.sync.dma_start(b1_s[:], b1[:])
            wp_s = cp.tile([DIM, 1], F32, tag="wp_s")
            nc.sync.dma_start(wp_s[:], wp[:])
            bp_s = cp.tile([1, 1], F32, tag="bp_s")
            nc.sync.dma_start(bp_s[:], bp[:])
            sel_s = cp.tile([128, MPC], F32, tag="sel_s")
            nc.sync.dma_start(sel_s[:], sel[:])
            fpi_all_s = cp.tile([128, N // 16], I16, tag="fpi_all_s")
            nc.sync.dma_start(fpi_all_s[:], fpi_all[:])
            fpi_own_s = cp.tile([128, APC // 16], I16, tag="fpi_own_s")
            nc.sync.dma_start(fpi_own_s[:], fpi_own[:])
            bidx_s = cp.tile([128, APC * N // 16], I16, tag="bidx_s")
            nc.sync.dma_start(bidx_s[:], bidx[:])
            at_s = cp.tile([128, NCH, 128], BF, tag="at_s")
            for j in range(NCH):
                nc.sync.dma_start(at_s[:, j, :], at[j * 128:(j + 1) * 128, :])

            # ---- MPS stage ---------------------------------------------
            # mps0T: [c, b] bf16 for all 1024 atoms (replicated compute)
            mps0T = cp.tile([128, 1, N], BF, tag="mps0T")
            nc.gpsimd.dma_gather(
                out_ap=mps0T[:], in_ap=e_fp[:], idxs_ap=fpi_all_s[:],
                num_idxs=N, num_idxs_reg=N, elem_size=DIM, transpose=True,
                single_packet=False)
            # mps0 for own rows, [c, a_own]
            mps0oT = cp.tile([128, 1, APC], BF, tag="mps0oT")
            nc.gpsimd.dma_gather(
                out_ap=mps0oT[:], in_ap=e_fp[:], idxs_ap=fpi_own_s[:],
                num_idxs=APC, num_idxs_reg=APC, elem_size=DIM, transpose=True,
                single_packet=False)

            # contri[b, c'] = relu(mps0 @ W_fp + b_fp), chunked over b
            contri_s = cp.tile([128, NCH, DIM], BF, tag="contri_s")
            for j in range(NCH):
                cps = psB.tile([128, DIM], F32, tag="ps")
                nc.tensor.matmul(cps[:], lhsT=mps0T[:, 0, j * 128:(j + 1) * 128],
                                 rhs=wfp_s[:], start=True, stop=False)
                nc.tensor.matmul(cps[:], lhsT=onesbf_s[:], rhs=bfp_s[:],
                                 start=False, stop=True)
                nc.vector.tensor_scalar_max(contri_s[:, j, :], cps[:], 0.0)

            # mps_own[a, c] = mps0_own + A[own rows] @ contri   (dense)
            mps_ps = psB.tile([128, DIM], F32, tag="ps")
            for j in range(NCH):
                nc.tensor.matmul(mps_ps[:], lhsT=at_s[:, j, :],
                                 rhs=contri_s[:, j, :],
                                 start=(j == 0), stop=False)
            nc.tensor.matmul(mps_ps[:], lhsT=mps0oT[:, 0, :], rhs=idbf_s[:],
                             start=False, stop=True)
            mps_own = cp.tile([128, DIM], F32, tag="mps_own")
            nc.vector.tensor_copy(mps_own[:], mps_ps[:])

            # l2 normalize rows (free axis)
            nsq = smp.tile([128, 1], F32, tag="nsq")
            nscr = smp.tile([128, DIM], F32, tag="nscr")
            nc.scalar.activation(out=nscr[:], in_=mps_own[:],
                                 func=AFT.Square, accum_out=nsq[:])
            nrm = smp.tile([128, 1], F32, tag="nrm")
            nc.scalar.sqrt(nrm[:], nsq[:])
            nrm2 = smp.tile([128, 1], F32, tag="nrm2")
            nc.vector.tensor_scalar_max(nrm2[:], nrm[:], EPS)
            inv = smp.tile([128, 1], F32, tag="inv")
            nc.vector.reciprocal(inv[:], nrm2[:])
            mps_n = cp.tile([128, DIM], F32, tag="mps_n")  # [a_own, c]
            nc.vector.tensor_scalar_mul(mps_n[:], mps_own[:], inv[:])
            # transpose -> [c, a_own]
            mnt_ps = psB.tile([128, 128], F32, tag="ps")
            nc.tensor.transpose(mnt_ps[:], mps_n[:], idf_s[:])
            mps_nT = cp.tile([128, 128], F32, tag="mps_nT")
            nc.vector.tensor_copy(mps_nT[:], mnt_ps[:])

            # ---- main loop: mpo gather + ssq + diag matmuls ------------
            ssq = cp.tile([128, APC], F32, tag="ssq")   # [c, a_local]
            wT = cp.tile([128, APC], F32, tag="wT")     # [c, a_local]
            tmp_ps = psA.tile([128, N], F32, tag="tmp_ps")  # [c, b] accum

            nch_eff = int(stage[1:]) if stage.startswith("g") else NCH
            for ch in range(nch_eff):
                slab = slabp.tile([128, 1, ACH * N], BF, tag="slab")
                # 4 gather calls of 4096 idxs each (SWDGE ring carveout
                # holds ~1000 descriptors; 258/call leaves pipelining room)
                for q in range(GCALLS):
                    i0 = ch * (ACH * N // 16) + q * (GIDX // 16)
                    nc.gpsimd.dma_gather(
                        out_ap=slab[:, :, q * GIDX:(q + 1) * GIDX],
                        in_ap=e_bond[:],
                        idxs_ap=bidx_s[:, i0:i0 + GIDX // 16],
                        num_idxs=GIDX, num_idxs_reg=GIDX,
                        elem_size=DIM, transpose=True, single_packet=False)

                if stage == "gather":
                    gdump = smp.tile([128, GCALLS], BF, tag="gdump")
                    for q in range(GCALLS):
                        nc.vector.tensor_copy(gdump[:, q:q + 1],
                                              slab[:, 0, q * GIDX:q * GIDX + 1])
                    continue
                # ssq: first DVE_J rows on DVE (square + tree-add),
                # the rest on ACT (Square + accum_out).
                if DVE_J > 0:
                    sq = scrp.tile([128, DVE_J, N], BF, tag="dscr")
                    nc.vector.tensor_mul(
                        sq.rearrange("p j n -> p (j n)"),
                        slab[:, 0, :DVE_J * N], slab[:, 0, :DVE_J * N])
                    # tree-add over b within each row
                    t1 = scrp.tile([128, DVE_J, N // 2], BF, tag="tr1")
                    t2 = scrp.tile([128, DVE_J, N // 4], BF, tag="tr2")
                    nc.vector.tensor_add(t1[:], sq[:, :, :N // 2],
                                         sq[:, :, N // 2:])
                    nc.vector.tensor_add(t2[:], t1[:, :, :N // 4],
                                         t1[:, :, N // 4:])
                    lvls = [t2]
                    w_ = N // 4
                    while w_ > 2:
                        w_ //= 2
                        nxt = scrp.tile([128, DVE_J, w_], BF,
                                        tag=f"tr{w_}")
                        nc.vector.tensor_add(nxt[:], lvls[-1][:, :, :w_],
                                             lvls[-1][:, :, w_:])
                        lvls.append(nxt)
                    # final level -> f32 ssq columns
                    last = lvls[-1]
                    nc.vector.tensor_add(
                        ssq[:, ch * ACH:ch * ACH + DVE_J],
                        last[:, :, 0], last[:, :, 1])
                for j in range(DVE_J, ACH):
                    al = ch * ACH + j
                    scr = scrp.tile([128, N], BF, tag="ascr")
                    nc.scalar.activation(
                        out=scr[:], in_=slab[:, 0, j * N:(j + 1) * N],
                        func=AFT.Square, accum_out=ssq[:, al:al + 1])

                if stage == "ssq":
                    continue
                # w for this chunk: w[c, a] = mps_nT / max(sqrt(ssq), eps)
                c0, c1 = ch * ACH, (ch + 1) * ACH
                st = smp.tile([128, ACH], F32, tag="st")
                nc.scalar.sqrt(st[:], ssq[:, c0:c1])
                st2 = smp.tile([128, ACH], F32, tag="st2")
                nc.vector.tensor_scalar_max(st2[:], st[:], EPS)
                sti = smp.tile([128, ACH], F32, tag="sti")
                nc.vector.reciprocal(sti[:], st2[:])
                nc.vector.tensor_mul(wT[:, c0:c1], mps_nT[:, c0:c1], sti[:])

                # tmp[c, b] += diag(w_a) @ slab_a
                for j in range(ACH):
                    al = ch * ACH + j
                    diag = diagp.tile([128, 128], BF, tag="diag")
                    nc.vector.tensor_scalar_mul(diag[:], idbf_s[:],
                                                wT[:, al:al + 1])
                    nc.tensor.matmul(
                        tmp_ps[:, 0:512], lhsT=diag[:],
                        rhs=slab[:, 0, j * N:j * N + 512],
                        start=(al == 0), stop=(al == nch_eff * ACH - 1),
                        skip_group_check=True)
                    nc.tensor.matmul(
                        tmp_ps[:, 512:1024], lhsT=diag[:],
                        rhs=slab[:, 0, j * N + 512:(j + 1) * N],
                        start=(al == 0), stop=(al == nch_eff * ACH - 1),
                        skip_group_check=True)

            if stage in ("gather", "ssq", "mm"):  # early-exit debug stages
                # debug early-exit: emit a token output and stop
                dbg = smp.tile([MPC, 1], F32, tag="dbg")
                if stage == "gather":
                    nc.vector.tensor_copy(dbg[:], slab[:MPC, 0, 0:1])
                elif stage == "ssq":
                    nc.vector.tensor_copy(dbg[:], ssq[:MPC, 0:1])
                else:
                    tmp_dbg = cp.tile([128, N], F32, tag="tmp_dbg")
                    nc.vector.tensor_copy(tmp_dbg[:], tmp_ps[:])
                    nc.vector.tensor_copy(dbg[:], tmp_dbg[:MPC, 0:1])
                nc.sync.dma_start(out[:], dbg[:])
                nc.compile()
                return nc

            # ---- tmp -> b-major -> ReduceScatter -----------------------
            tmp_s = cp.tile([128, N], F32, tag="tmp_s")
            nc.vector.tensor_copy(tmp_s[:], tmp_ps[:])
            for j in range(NCH):
                tps = psB.tile([128, 128], F32, tag="ps")
                nc.tensor.transpose(tps[:], tmp_s[:, j * 128:(j + 1) * 128],
                                    idf_s[:])
                tts = smp.tile([128, 128], F32, tag="tts")
                nc.vector.tensor_copy(tts[:], tps[:])
                nc.sync.dma_start(cc_in[j * 128:(j + 1) * 128, :], tts[:])

            nc.gpsimd.collective_compute(
                "ReduceScatter", AOT.add,
                replica_groups=[list(range(NCORES))],
                ins=[cc_in[:]], outs=[cc_out[:]])

            # ---- finale: tn + MLP (own 8 molecules) --------------------
            tro = smp.tile([128, DIM], F32, tag="tro")  # [b_own, c]
            nc.sync.dma_start(tro[:], cc_out[:])
            prod = smp.tile([128, DIM], F32, tag="prod")
            nc.vector.tensor_mul(prod[:], tro[:], mps_n[:])
            tn_ps = psB.tile([MPC, DIM], F32, tag="ps")
            nc.tensor.matmul(tn_ps[:], lhsT=sel_s[:], rhs=prod[:],
                             start=True, stop=True)
            tn_s = smp.tile([MPC, DIM], F32, tag="tn_s")
            nc.vector.tensor_copy(tn_s[:], tn_ps[:])
            tnT_ps = psB.tile([128, MPC], F32, tag="ps")
            nc.tensor.transpose(tnT_ps[:], tn_s[:], idf_s[:MPC, :MPC])
            tnT_s = smp.tile([128, MPC], F32, tag="tnT_s")
            nc.vector.tensor_copy(tnT_s[:], tnT_ps[:])

            x0_ps = psB.tile([MPC, DIM], F32, tag="ps")
            nc.tensor.matmul(x0_ps[:], lhsT=tnT_s[:], rhs=w0_s[:],
                             start=True, stop=False)
            nc.tensor.matmul(x0_ps[:], lhsT=onesf_s[:, :MPC], rhs=b0_s[:],
                             start=False, stop=True)
            x0_s = smp.tile([MPC, DIM], F32, tag="x0_s")
            nc.vector.tensor_scalar_max(x0_s[:], x0_ps[:], 0.0)
            x0T_ps = psB.tile([128, MPC], F32, tag="ps")
            nc.tensor.transpose(x0T_ps[:], x0_s[:], idf_s[:MPC, :MPC])
            x0T_s = smp.tile([128, MPC], F32, tag="x0T_s")
            nc.vector.tensor_copy(x0T_s[:], x0T_ps[:])

            x1_ps = psB.tile([MPC, DIM], F32, tag="ps")
            nc.tensor.matmul(x1_ps[:], lhsT=x0T_s[:], rhs=w1_s[:],
                             start=True, stop=False)
            nc.tensor.matmul(x1_ps[:], lhsT=onesf_s[:, :MPC], rhs=b1_s[:],
                             start=False, stop=True)
            x1_s = smp.tile([MPC, DIM], F32, tag="x1_s")
            nc.vector.tensor_scalar_max(x1_s[:], x1_ps[:], 0.0)
            x1T_ps = psB.tile([128, MPC], F32, tag="ps")
            nc.tensor.transpose(x1T_ps[:], x1_s[:], idf_s[:MPC, :MPC])
            x1T_s = smp.tile([128, MPC], F32, tag="x1T_s")
            nc.vector.tensor_copy(x1T_s[:], x1T_ps[:])

            y_ps = psB.tile([MPC, 1], F32, tag="ps")
            nc.tensor.matmul(y_ps[:], lhsT=x1T_s[:], rhs=wp_s[:],
                             start=True, stop=False)
            nc.tensor.matmul(y_ps[:], lhsT=onesf_s[:, :MPC], rhs=bp_s[:, :1],
                             start=False, stop=True)
            y_s = smp.tile([MPC, 1], F32, tag="y_s")
            nc.vector.tensor_copy(y_s[:], y_ps[:])
            nc.sync.dma_start(out[:], y_s[:])

    nc.compile()
    return nc




def build_fast():
    """Fast path: bond_index verified block-diagonal with off-block == 0.

    Gathers only the 2048 in-block E_bond rows per core; the off-block
    contribution (all index 0 -> row T0) is added analytically:
      ssq[a,c]   = ssq_in[a,c] + 1008*T0[c]^2
      tmp[c,b]   = tmp_in[c,b] + T0[c]*(gwsum[c] - wmol[c, mol(b)])
    gwsum is a 512-byte AllReduce of per-core w column sums; everything
    else is core-local, so no ReduceScatter of tmp is needed.
    """
    nc = bacc.Bacc("TRN2", target_bir_lowering=False, debug=False,
                   num_devices=NCORES)

    e_bond = nc.declare_dram_parameter("e_bond", [N_BOND, DIM], BF, False)
    e_fp = nc.declare_dram_parameter("e_fp", [N_FP, DIM], BF, False)
    fpi_all = nc.declare_dram_parameter("fpi_all", [128, N // 16], I16, False)
    fpi_own = nc.declare_dram_parameter("fpi_own", [128, APC // 16], I16, False)
    ibx = nc.declare_dram_parameter("ibx", [128, APC * NA // 16], I16, False)
    at = nc.declare_dram_parameter("at", [N, APC], BF, False)
    wfp = nc.declare_dram_parameter("wfp", [DIM, DIM], BF, False)
    bfp = nc.declare_dram_parameter("bfp", [1, DIM], BF, False)
    w0 = nc.declare_dram_parameter("w0", [DIM, DIM], F32, False)
    b0 = nc.declare_dram_parameter("b0", [1, DIM], F32, False)
    w1 = nc.declare_dram_parameter("w1", [DIM, DIM], F32, False)
    b1 = nc.declare_dram_parameter("b1", [1, DIM], F32, False)
    wp = nc.declare_dram_parameter("wp", [DIM, 1], F32, False)
    bp = nc.declare_dram_parameter("bp", [1, 1], F32, False)
    idbf = nc.declare_dram_parameter("idbf", [128, 128], BF, False)
    idf = nc.declare_dram_parameter("idf", [128, 128], F32, False)
    onesbf = nc.declare_dram_parameter("onesbf", [1, 128], BF, False)
    onesf = nc.declare_dram_parameter("onesf", [1, 128], F32, False)
    t0col = nc.declare_dram_parameter("t0col", [128, 1], F32, False)
    out = nc.declare_dram_parameter("out", [MPC, 1], F32, True)

    cc_in = nc.dram_tensor("cc_in", [128, 1], F32)
    cc_out = nc.dram_tensor("cc_out", [128, 1], F32, addr_space="Shared")

    NIB = APC * NA  # 2048 in-block indices per core

    with tile.TileContext(nc) as tc:
        with (
            tc.tile_pool(name="const", bufs=1) as cp,
            tc.tile_pool(name="small", bufs=2) as smp,
            tc.tile_pool(name="psB", bufs=3, space="PSUM") as psB,
        ):
            # ---- constants -------------------------------------------
            wfp_s = cp.tile([DIM, DIM], BF, tag="wfp_s")
            nc.sync.dma_start(wfp_s[:], wfp[:])
            idbf_s = cp.tile([128, 128], BF, tag="idbf_s")
            nc.sync.dma_start(idbf_s[:], idbf[:])
            idf_s = cp.tile([128, 128], F32, tag="idf_s")
            nc.sync.dma_start(idf_s[:], idf[:])
            onesbf_s = cp.tile([1, 128], BF, tag="onesbf_s")
            nc.sync.dma_start(onesbf_s[:], onesbf[:])
            onesf_s = cp.tile([1, 128], F32, tag="onesf_s")
            nc.sync.dma_start(onesf_s[:], onesf[:])
            bfp_s = cp.tile([1, DIM], BF, tag="bfp_s")
            nc.sync.dma_start(bfp_s[:], bfp[:])
            w0_s = cp.tile([DIM, DIM], F32, tag="w0_s")
            nc.sync.dma_start(w0_s[:], w0[:])
            b0_s = cp.tile([1, DIM], F32, tag="b0_s")
            nc.sync.dma_start(b0_s[:], b0[:])
            w1_s = cp.tile([DIM, DIM], F32, tag="w1_s")
            nc.sync.dma_start(w1_s[:], w1[:])
            b1_s = cp.tile([1, DIM], F32, tag="b1_s")
            nc.sync.dma_start(b1_s[:], b1[:])
            wp_s = cp.tile([DIM, 1], F32, tag="wp_s")
            nc.sync.dma_start(wp_s[:], wp[:])
            bp_s = cp.tile([1, 1], F32, tag="bp_s")
            nc.sync.dma_start(bp_s[:], bp[:])
            fpi_all_s = cp.tile([128, N // 16], I16, tag="fpi_all_s")
            nc.sync.dma_start(fpi_all_s[:], fpi_all[:])
            fpi_own_s = cp.tile([128, APC // 16], I16, tag="fpi_own_s")
            nc.sync.dma_start(fpi_own_s[:], fpi_own[:])
            ibx_s = cp.tile([128, NIB // 16], I16, tag="ibx_s")
            nc.sync.dma_start(ibx_s[:], ibx[:])
            at_s = cp.tile([128, NCH, 128], BF, tag="at_s")
            for j in range(NCH):
                nc.sync.dma_start(at_s[:, j, :], at[j * 128:(j + 1) * 128, :])
            t0_s = cp.tile([128, 1], F32, tag="t0_s")
            nc.sync.dma_start(t0_s[:], t0col[:])

            # ---- gathers (start immediately, overlap mps stage) ------
            inb = cp.tile([128, 1, NIB], BF, tag="inb")  # [c, (m,a,n)]
            nc.gpsimd.dma_gather(
                out_ap=inb[:], in_ap=e_bond[:], idxs_ap=ibx_s[:],
                num_idxs=NIB, num_idxs_reg=NIB, elem_size=DIM,
                transpose=True, single_packet=False)
            mps0T = cp.tile([128, 1, N], BF, tag="mps0T")
            nc.gpsimd.dma_gather(
                out_ap=mps0T[:], in_ap=e_fp[:], idxs_ap=fpi_all_s[:],
                num_idxs=N, num_idxs_reg=N, elem_size=DIM, transpose=True,
                single_packet=False)
            mps0oT = cp.tile([128, 1, APC], BF, tag="mps0oT")
            nc.gpsimd.dma_gather(
                out_ap=mps0oT[:], in_ap=e_fp[:], idxs_ap=fpi_own_s[:],
                num_idxs=APC, num_idxs_reg=APC, elem_size=DIM,
                transpose=True, single_packet=False)

            # ---- MPS stage (identical to slow path) ------------------
            contri_s = cp.tile([128, NCH, DIM], BF, tag="contri_s")
            for j in range(NCH):
                cps = psB.tile([128, DIM], F32, tag="ps")
                nc.tensor.matmul(cps[:], lhsT=mps0T[:, 0, j * 128:(j + 1) * 128],
                                 rhs=wfp_s[:], start=True, stop=False)
                nc.tensor.matmul(cps[:], lhsT=onesbf_s[:], rhs=bfp_s[:],
                                 start=False, stop=True)
                nc.vector.tensor_scalar_max(contri_s[:, j, :], cps[:], 0.0)

            mps_ps = psB.tile([128, DIM], F32, tag="ps")
            for j in range(NCH):
                nc.tensor.matmul(mps_ps[:], lhsT=at_s[:, j, :],
                                 rhs=contri_s[:, j, :],
                                 start=(j == 0), stop=False)
            nc.tensor.matmul(mps_ps[:], lhsT=mps0oT[:, 0, :], rhs=idbf_s[:],
                             start=False, stop=True)
            mps_own = cp.tile([128, DIM], F32, tag="mps_own")
            nc.vector.tensor_copy(mps_own[:], mps_ps[:])

            nsq = smp.tile([128, 1], F32, tag="nsq")
            nscr = smp.tile([128, DIM], F32, tag="nscr")
            nc.scalar.activation(out=nscr[:], in_=mps_own[:],
                                 func=AFT.Square, accum_out=nsq[:])
            nrm = smp.tile([128, 1], F32, tag="nrm")
            nc.scalar.sqrt(nrm[:], nsq[:])
            nrm2 = smp.tile([128, 1], F32, tag="nrm2")
            nc.vector.tensor_scalar_max(nrm2[:], nrm[:], EPS)
            inv = smp.tile([128, 1], F32, tag="inv")
            nc.vector.reciprocal(inv[:], nrm2[:])
            mps_n = cp.tile([128, DIM], F32, tag="mps_n")  # [a_own, c]
            nc.vector.tensor_scalar_mul(mps_n[:], mps_own[:], inv[:])
            mnt_ps = psB.tile([128, 128], F32, tag="ps")
            nc.tensor.transpose(mnt_ps[:], mps_n[:], idf_s[:])
            mps_nT = cp.tile([128, 128], F32, tag="mps_nT")  # [c, a_own]
            nc.vector.tensor_copy(mps_nT[:], mnt_ps[:])

            # ---- ssq from in-block slab + analytic T0 part -----------
            sq = cp.tile([128, NIB], BF, tag="sq")
            nc.vector.tensor_mul(sq[:], inb[:, 0, :], inb[:, 0, :])
            ssq_in = cp.tile([128, APC], F32, tag="ssq_in")  # [c, (m,a)]
            nc.vector.reduce_sum(
                ssq_in.rearrange("p (g o) -> p g o", o=1),
                sq.rearrange("p (g n) -> p g n", n=NA),
                axis=mybir.AxisListType.X)
            # t0sq1008 = 1008 * T0^2
            t0sq = smp.tile([128, 1], F32, tag="t0sq")
            nc.scalar.activation(out=t0sq[:], in_=t0_s[:], func=AFT.Square)
            t0sqk = smp.tile([128, 1], F32, tag="t0sqk")
            nc.vector.tensor_scalar_mul(t0sqk[:], t0sq[:], float(N - NA))
            ssq = cp.tile([128, APC], F32, tag="ssq")
            nc.vector.tensor_scalar(ssq[:], ssq_in[:], t0sqk[:], None,
                                    op0=AOT.add)

            # ---- w = mps_nT / max(sqrt(ssq), eps);  [c, a_local] ------
            st = smp.tile([128, APC], F32, tag="st")
            nc.scalar.sqrt(st[:], ssq[:])
            st2 = smp.tile([128, APC], F32, tag="st2")
            nc.vector.tensor_scalar_max(st2[:], st[:], EPS)
            sti = smp.tile([128, APC], F32, tag="sti")
            nc.vector.reciprocal(sti[:], st2[:])
            wT = cp.tile([128, APC], F32, tag="wT")
            nc.vector.tensor_mul(wT[:], mps_nT[:], sti[:])

            # ---- gwsum AllReduce (512 B) ------------------------------
            wsum = smp.tile([128, 1], F32, tag="wsum")
            nc.vector.reduce_sum(wsum[:], wT[:], axis=mybir.AxisListType.X)
            nc.sync.dma_start(cc_in[:], wsum[:])
            nc.gpsimd.collective_compute(
                "AllReduce", AOT.add, replica_groups=[list(range(NCORES))],
                ins=[cc_in[:]], outs=[cc_out[:]])
            gw = smp.tile([128, 1], F32, tag="gw")
            nc.sync.dma_start(gw[:], cc_out[:])

            # wmol[c, m] = sum of w over mol m's 16 atoms
            wmol = smp.tile([128, MPC], F32, tag="wmol")
            nc.vector.reduce_sum(
                wmol.rearrange("p (g o) -> p g o", o=1),
                wT.rearrange("p (g n) -> p g n", n=NA),
                axis=mybir.AxisListType.X)
            # tadd[c, m] = T0[c] * (gwsum[c] - wmol[c, m])
            d1 = smp.tile([128, MPC], F32, tag="d1")
            nc.vector.tensor_scalar(d1[:], wmol[:], gw[:], None,
                                    op0=AOT.subtract)  # wmol - gw
            nt0 = smp.tile([128, 1], F32, tag="nt0")
            nc.vector.tensor_scalar_mul(nt0[:], t0_s[:], -1.0)
            tadd = smp.tile([128, MPC], F32, tag="tadd")
            nc.vector.tensor_scalar(tadd[:], d1[:], nt0[:], None,
                                    op0=AOT.mult)  # (gw-wmol)*T0

            # ---- tmp_in[c, b_local] = sum_a w[a,c]*inb[c,(m,a,n)] -----
            prod = cp.tile([128, NIB], F32, tag="prod")
            for m in range(MPC):
                for a_l in range(NA):
                    al = m * NA + a_l
                    csl = slice(al * NA, (al + 1) * NA)
                    nc.vector.tensor_scalar_mul(prod[:, csl],
                                                inb[:, 0, csl],
                                                wT[:, al:al + 1])
            tmp_in = cp.tile([128, APC], F32, tag="tmp_in")  # [c,(m,n)]
            nc.vector.reduce_sum(
                tmp_in.rearrange("p (m n o) -> p m n o", n=NA, o=1),
                prod.rearrange("p (m a n) -> p m n a", a=NA, n=NA),
                axis=mybir.AxisListType.X)

            # tmp_own[c, (m,n)] = tmp_in + tadd[c, m]
            tmp_own = cp.tile([128, APC], F32, tag="tmp_own")
            for m in range(MPC):
                msl = slice(m * NA, (m + 1) * NA)
                nc.vector.tensor_scalar(tmp_own[:, msl], tmp_in[:, msl],
                                        tadd[:, m:m + 1], None, op0=AOT.add)

            # ---- finale in [c, b] layout ------------------------------
            fprod = smp.tile([128, APC], F32, tag="fprod")
            nc.vector.tensor_mul(fprod[:], tmp_own[:], mps_nT[:])
            tnT = smp.tile([128, MPC], F32, tag="tnT")  # [d, m]
            nc.vector.reduce_sum(
                tnT.rearrange("p (g o) -> p g o", o=1),
                fprod.rearrange("p (g n) -> p g n", n=NA),
                axis=mybir.AxisListType.X)

            x0_ps = psB.tile([MPC, DIM], F32, tag="ps")
            nc.tensor.matmul(x0_ps[:], lhsT=tnT[:], rhs=w0_s[:],
                             start=True, stop=False)
            nc.tensor.matmul(x0_ps[:], lhsT=onesf_s[:, :MPC], rhs=b0_s[:],
                             start=False, stop=True)
            x0_s = smp.tile([MPC, DIM], F32, tag="x0_s")
            nc.vector.tensor_scalar_max(x0_s[:], x0_ps[:], 0.0)
            x0T_ps = psB.tile([128, MPC], F32, tag="ps")
            nc.tensor.transpose(x0T_ps[:], x0_s[:], idf_s[:MPC, :MPC])
            x0T_s = smp.tile([128, MPC], F32, tag="x0T_s")
            nc.vector.tensor_copy(x0T_s[:], x0T_ps[:])

            x1_ps = psB.tile([MPC, DIM], F32, tag="ps")
            nc.tensor.matmul(x1_ps[:], lhsT=x0T_s[:], rhs=w1_s[:],
                             start=True, stop=False)
            nc.tensor.matmul(x1_ps[:], lhsT=onesf_s[:, :MPC], rhs=b1_s[:],
                             start=False, stop=True)
            x1_s = smp.tile([MPC, DIM], F32, tag="x1_s")
            nc.vector.tensor_scalar_max(x1_s[:], x1_ps[:], 0.0)
            x1T_ps = psB.tile([128, MPC], F32, tag="ps")
            nc.tensor.transpose(x1T_ps[:], x1_s[:], idf_s[:MPC, :MPC])
            x1T_s = smp.tile([128, MPC], F32, tag="x1T_s")
            nc.vector.tensor_copy(x1T_s[:], x1T_ps[:])

            y_ps = psB.tile([MPC, 1], F32, tag="ps")
            nc.tensor.matmul(y_ps[:], lhsT=x1T_s[:], rhs=wp_s[:],
                             start=True, stop=False)
            nc.tensor.matmul(y_ps[:], lhsT=onesf_s[:, :MPC], rhs=bp_s[:, :1],
                             start=False, stop=True)
            y_s = smp.tile([MPC, 1], F32, tag="y_s")
            nc.vector.tensor_copy(y_s[:], y_ps[:])
            nc.sync.dma_start(out[:], y_s[:])

    nc.compile()
    return nc


def make_in_maps_fast(fingerprints, adjacency, bond_index, E_fp, E_bond,
                      W_fp, b_fp, W_out0, b_out0, W_out1, b_out1, W_prop,
                      b_prop):
    e_bond_bf = np.ascontiguousarray(E_bond.astype(BF16))
    e_fp_bf = np.ascontiguousarray(E_fp.astype(BF16))
    wfp_bf = np.ascontiguousarray(W_fp.astype(BF16))
    bfp_bf = b_fp.astype(BF16).reshape(1, DIM)
    w0_f = np.ascontiguousarray(W_out0.astype(np.float32))
    b0_f = b_out0.astype(np.float32).reshape(1, DIM)
    w1_f = np.ascontiguousarray(W_out1.astype(np.float32))
    b1_f = b_out1.astype(np.float32).reshape(1, DIM)
    wp_f = np.ascontiguousarray(W_prop.astype(np.float32))
    bp_f = b_prop.astype(np.float32).reshape(1, 1)
    idbf = np.eye(128, dtype=BF16)
    idf = np.eye(128, dtype=np.float32)
    onesbf = np.ones((1, 128), dtype=BF16)
    onesf = np.ones((1, 128), dtype=np.float32)
    # T0 as the bf16-rounded row (matches the gathered slab precision)
    t0 = E_bond[0].astype(BF16).astype(np.float32).reshape(128, 1)
    fpi_all = _wrap_idx16(fingerprints)

    in_maps = []
    for k in range(NCORES):
        rows = slice(k * APC, (k + 1) * APC)
        at_k = np.ascontiguousarray(adjacency[rows, :].T.astype(BF16))
        fpi_own = _wrap_idx16(fingerprints[rows])
        # in-block indices, order (m, a_local, n)
        ib = np.empty(APC * NA, dtype=np.int16)
        pos = 0
        for m in range(MPC):
            gm = k * MPC + m
            blk = bond_index[gm * NA:(gm + 1) * NA, gm * NA:(gm + 1) * NA]
            ib[pos:pos + NA * NA] = blk.astype(np.int16).reshape(-1)
            pos += NA * NA
        in_maps.append({
            "e_bond": e_bond_bf, "e_fp": e_fp_bf,
            "fpi_all": fpi_all, "fpi_own": fpi_own,
            "ibx": _wrap_idx16(ib),
            "at": at_k, "wfp": wfp_bf, "bfp": bfp_bf,
            "w0": w0_f, "b0": b0_f, "w1": w1_f, "b1": b1_f,
            "wp": wp_f, "bp": bp_f,
            "idbf": idbf, "idf": idf, "onesbf": onesbf, "onesf": onesf,
            "t0col": t0,
        })
    return in_maps


def _offblock_is_zero(bond_index):
    mol = np.arange(N) // NA
    block = mol[:, None] == mol[None, :]
    return bool(np.all(np.asarray(bond_index)[~block] == 0))


_NC_CACHE = {}


def _get_nc(fast):
    key = "fast" if fast else "full"
    if key not in _NC_CACHE:
        _NC_CACHE[key] = build_fast() if fast else build_nc()
    return _NC_CACHE[key]


def make_in_maps(fingerprints, adjacency, bond_index, E_fp, E_bond, W_fp,
                 b_fp, W_out0, b_out0, W_out1, b_out1, W_prop, b_prop):
    e_bond_bf = np.ascontiguousarray(E_bond.astype(BF16))
    e_fp_bf = np.ascontiguousarray(E_fp.astype(BF16))
    wfp_bf = np.ascontiguousarray(W_fp.astype(BF16))
    bfp_bf = b_fp.astype(BF16).reshape(1, DIM)
    w0_f = np.ascontiguousarray(W_out0.astype(np.float32))
    b0_f = b_out0.astype(np.float32).reshape(1, DIM)
    w1_f = np.ascontiguousarray(W_out1.astype(np.float32))
    b1_f = b_out1.astype(np.float32).reshape(1, DIM)
    wp_f = np.ascontiguousarray(W_prop.astype(np.float32))
    bp_f = b_prop.astype(np.float32).reshape(1, 1)
    idbf = np.eye(128, dtype=BF16)
    idf = np.eye(128, dtype=np.float32)
    onesbf = np.ones((1, 128), dtype=BF16)
    onesf = np.ones((1, 128), dtype=np.float32)
    # molecule-sum selector: sel[b, m] = 1 if b // 16 == m
    sel = np.zeros((128, MPC), dtype=np.float32)
    for m in range(MPC):
        sel[m * NA:(m + 1) * NA, m] = 1.0

    fpi_all = _wrap_idx16(fingerprints)

    in_maps = []
    for k in range(NCORES):
        rows = slice(k * APC, (k + 1) * APC)
        # bond idx: wrapped per gather call (each call wraps its own idxs)
        flat = bond_index[rows, :].astype(np.int16).reshape(
            NCH * GCALLS, GIDX)
        bidx = np.concatenate(
            [_wrap_idx16(flat[c]) for c in range(NCH * GCALLS)],
            axis=1)  # [128, 8192]
        at_k = np.ascontiguousarray(adjacency[rows, :].T.astype(BF16))
        fpi_own = _wrap_idx16(fingerprints[rows])
        in_maps.append({
            "e_bond": e_bond_bf, "e_fp": e_fp_bf,
            "fpi_all": fpi_all, "fpi_own": fpi_own, "bidx": bidx,
            "at": at_k, "wfp": wfp_bf, "bfp": bfp_bf,
            "w0": w0_f, "b0": b0_f, "w1": w1_f, "b1": b1_f,
            "wp": wp_f, "bp": bp_f,
            "idbf": idbf, "idf": idf, "onesbf": onesbf, "onesf": onesf,
            "sel": sel,
        })
    return in_maps


def run(inputs, trace=False, **kw):
    fast = _offblock_is_zero(inputs["bond_index"])
    nc = _get_nc(fast)
    in_maps = make_in_maps_fast(**inputs) if fast else make_in_maps(**inputs)
    res = run_bass_kernel_spmd(nc, in_maps, core_ids=list(range(NCORES)),
                               trace=trace, **kw)
    out = np.concatenate([res.results[k]["out"] for k in range(NCORES)],
                         axis=0).astype(np.float32)
    return out, res


def kernel(**inputs):
    out, _ = run(inputs, trace=False)
    return out


# revision 24
# speedup vs baseline: 19.6740x; 1.2890x over previous
"""Trainium2 (8 NeuronCores) Bass kernel for the GNN message-passing module.

Math (reference):
  mps  = E_fp[fingerprints]                       [N, d]
  mps  = l2norm_rows(mps + A @ relu(mps @ W_fp + b_fp))
  mpo  = l2norm_axis1(E_bond[bond_index])         [N, N, d] (norm over 2nd atom axis)
  tmp[c,b] = sum_a mps[a,c] * mpo[a,b,c]
  tn[m,d]  = sum_n tmp[d, m*16+n] * mps[m*16+n, d]
  out  = relu(relu(tn@W0+b0)@W1+b1) @ Wp + bp     [B, 1]

Sharding: atom axis a (1024) split 128/core across 8 cores.  Each core
gathers its 128 rows of mpo (bf16, transposed layout [d, a*b]), computes
ssq over b (DVE tensor_tensor_reduce + ACT Square/accum split), and
accumulates tmp[c,b] += diag(w_a) @ slab_a on the TensorEngine.  Per-core
partial tmp is transposed to b-major and ReduceScattered so core k
receives exactly the summed tmp columns of its own 8 molecules; the tiny
MLP finale runs per-core and outputs [8, 1], concatenated on the host.
"""

import sys

for _p in ("/opt/trn_rl_repo",):
    if _p not in sys.path:
        sys.path.insert(0, _p)

import numpy as np
import ml_dtypes

# Some images lack antenv.axon_hooks; bass_utils imports it unguarded when
# trace=True. Provide a shim so the import never crashes (hook stays None ->
# tracing is skipped gracefully unless a real hook is registered).
try:
    import antenv.axon_hooks  # noqa: F401
except ImportError:
    import types as _types

    import antenv as _antenv

    _m = _types.ModuleType("antenv.axon_hooks")
    _m._hook = None

    def _set_hook(h):
        _m._hook = h

    def _get_hook():
        return _m._hook

    _m.set_axon_ntff_profile_hook = _set_hook
    _m.get_axon_ntff_profile_hook = _get_hook
    sys.modules["antenv.axon_hooks"] = _m
    _antenv.axon_hooks = _m

import concourse.bacc as bacc
import concourse.mybir as mybir
import concourse.tile as tile
from concourse.bass_utils import run_bass_kernel_spmd

BF16 = ml_dtypes.bfloat16

NCORES = 8
N = 1024          # total atoms
DIM = 128
B = 64            # molecules
NA = 16           # atoms per molecule
APC = N // NCORES  # atoms per core = 128
MPC = B // NCORES  # molecules per core = 8
NCH = 8           # compute groups per core
ACH = APC // NCH  # a-rows per group = 16
GCALLS = 4        # gather calls per group (ring carveout limits a call
                  # to ~1000 descriptors = ~4K indices with pipelining room)
GIDX = ACH * N // GCALLS  # 4096 indices per gather call
N_FP = 10000
N_BOND = 10000
EPS = 1e-12
# Of each group's 16 a-rows, how many take the DVE (square + tree-add)
# ssq path; the rest go to ACT (Square + accum_out).
DVE_J = 7

F32 = mybir.dt.float32
BF = mybir.dt.bfloat16
I16 = mybir.dt.int16
AOT = mybir.AluOpType
AFT = mybir.ActivationFunctionType


def _wrap_idx16(flat):
    """SWDGE dma_gather index layout: idx i -> partition i%16, col i//16,
    replicated across the 8 Q7 cores (rows 16..127)."""
    flat = np.ascontiguousarray(flat.astype(np.int16))
    n = flat.shape[0]
    assert n % 16 == 0
    w = flat.reshape(n // 16, 16).T  # [16, n//16]
    return np.tile(w, (8, 1))        # [128, n//16]


def build_nc(stage="full"):
    nc = bacc.Bacc("TRN2", target_bir_lowering=False, debug=False,
                   num_devices=NCORES)

    e_bond = nc.declare_dram_parameter("e_bond", [N_BOND, DIM], BF, False)
    e_fp = nc.declare_dram_parameter("e_fp", [N_FP, DIM], BF, False)
    fpi_all = nc.declare_dram_parameter("fpi_all", [128, N // 16], I16, False)
    fpi_own = nc.declare_dram_parameter("fpi_own", [128, APC // 16], I16, False)
    bidx = nc.declare_dram_parameter("bidx", [128, APC * N // 16], I16, False)
    at = nc.declare_dram_parameter("at", [N, APC], BF, False)
    wfp = nc.declare_dram_parameter("wfp", [DIM, DIM], BF, False)
    bfp = nc.declare_dram_parameter("bfp", [1, DIM], BF, False)
    w0 = nc.declare_dram_parameter("w0", [DIM, DIM], F32, False)
    b0 = nc.declare_dram_parameter("b0", [1, DIM], F32, False)
    w1 = nc.declare_dram_parameter("w1", [DIM, DIM], F32, False)
    b1 = nc.declare_dram_parameter("b1", [1, DIM], F32, False)
    wp = nc.declare_dram_parameter("wp", [DIM, 1], F32, False)
    bp = nc.declare_dram_parameter("bp", [1, 1], F32, False)
    idbf = nc.declare_dram_parameter("idbf", [128, 128], BF, False)
    idf = nc.declare_dram_parameter("idf", [128, 128], F32, False)
    onesbf = nc.declare_dram_parameter("onesbf", [1, 128], BF, False)
    onesf = nc.declare_dram_parameter("onesf", [1, 128], F32, False)
    sel = nc.declare_dram_parameter("sel", [128, MPC], F32, False)
    out = nc.declare_dram_parameter("out", [MPC, 1], F32, True)

    cc_in = nc.dram_tensor("cc_in", [N, DIM], F32)
    cc_out = nc.dram_tensor("cc_out", [APC, DIM], F32)

    with tile.TileContext(nc) as tc:
        with (
            tc.tile_pool(name="const", bufs=1) as cp,
            tc.tile_pool(name="slab", bufs=2) as slabp,
            tc.tile_pool(name="scr", bufs=2) as scrp,
            tc.tile_pool(name="small", bufs=2) as smp,
            tc.tile_pool(name="diag", bufs=4) as diagp,
            tc.tile_pool(name="psA", bufs=1, space="PSUM") as psA,
            tc.tile_pool(name="psB", bufs=3, space="PSUM") as psB,
        ):
            # ---- constants to SBUF -------------------------------------
            wfp_s = cp.tile([DIM, DIM], BF, tag="wfp_s")
            nc.sync.dma_start(wfp_s[:], wfp[:])
            idbf_s = cp.tile([128, 128], BF, tag="idbf_s")
            nc.sync.dma_start(idbf_s[:], idbf[:])
            idf_s = cp.tile([128, 128], F32, tag="idf_s")
            nc.sync.dma_start(idf_s[:], idf[:])
            onesbf_s = cp.tile([1, 128], BF, tag="onesbf_s")
            nc.sync.dma_start(onesbf_s[:], onesbf[:])
            onesf_s = cp.tile([1, 128], F32, tag="onesf_s")
            nc.sync.dma_start(onesf_s[:], onesf[:])
            bfp_s = cp.tile([1, DIM], BF, tag="bfp_s")
            nc.sync.dma_start(bfp_s[:], bfp[:])
            w0_s = cp.tile([DIM, DIM], F32, tag="w0_s")
            nc.sync.dma_start(w0_s[:], w0[:])
            b0_s = cp.tile([1, DIM], F32, tag="b0_s")
            nc.sync.dma_start(b0_s[:], b0[:])
            w1_s = cp.tile([DIM, DIM], F32, tag="w1_s")
            nc.sync.dma_start(w1_s[:], w1[:])
            b1_s = cp.tile([1, DIM], F32, tag="b1_s")
            nc.sync.dma_start(b1_s[:], b1[:])
            wp_s = cp.tile([DIM, 1], F32, tag="wp_s")
            nc.sync.dma_start(wp_s[:], wp[:])
            bp_s = cp.tile([1, 1], F32, tag="bp_s")
            nc.sync.dma_start(bp_s[:], bp[:])
            sel_s = cp.tile([128, MPC], F32, tag="sel_s")
            nc.sync.dma_start(sel_s[:], sel[:])
            fpi_all_s = cp.tile([128, N // 16], I16, tag="fpi_all_s")
            nc.sync.dma_start(fpi_all_s[:], fpi_all[:])
            fpi_own_s = cp.tile([128, APC // 16], I16, tag="fpi_own_s")
            nc.sync.dma_start(fpi_own_s[:], fpi_own[:])
            bidx_s = cp.tile([128, APC * N // 16], I16, tag="bidx_s")
            nc.sync.dma_start(bidx_s[:], bidx[:])
            at_s = cp.tile([128, NCH, 128], BF, tag="at_s")
            for j in range(NCH):
                nc.sync.dma_start(at_s[:, j, :], at[j * 128:(j + 1) * 128, :])

            # ---- MPS stage ---------------------------------------------
            # mps0T: [c, b] bf16 for all 1024 atoms (replicated compute)
            mps0T = cp.tile([128, 1, N], BF, tag="mps0T")
            nc.gpsimd.dma_gather(
                out_ap=mps0T[:], in_ap=e_fp[:], idxs_ap=fpi_all_s[:],
                num_idxs=N, num_idxs_reg=N, elem_size=DIM, transpose=True,
                single_packet=False)
            # mps0 for own rows, [c, a_own]
            mps0oT = cp.tile([128, 1, APC], BF, tag="mps0oT")
            nc.gpsimd.dma_gather(
                out_ap=mps0oT[:], in_ap=e_fp[:], idxs_ap=fpi_own_s[:],
                num_idxs=APC, num_idxs_reg=APC, elem_size=DIM, transpose=True,
                single_packet=False)

            # contri[b, c'] = relu(mps0 @ W_fp + b_fp), chunked over b
            contri_s = cp.tile([128, NCH, DIM], BF, tag="contri_s")
            for j in range(NCH):
                cps = psB.tile([128, DIM], F32, tag="ps")
                nc.tensor.matmul(cps[:], lhsT=mps0T[:, 0, j * 128:(j + 1) * 128],
                                 rhs=wfp_s[:], start=True, stop=False)
                nc.tensor.matmul(cps[:], lhsT=onesbf_s[:], rhs=bfp_s[:],
                                 start=False, stop=True)
                nc.vector.tensor_scalar_max(contri_s[:, j, :], cps[:], 0.0)

            # mps_own[a, c] = mps0_own + A[own rows] @ contri   (dense)
            mps_ps = psB.tile([128, DIM], F32, tag="ps")
            for j in range(NCH):
                nc.tensor.matmul(mps_ps[:], lhsT=at_s[:, j, :],
                                 rhs=contri_s[:, j, :],
                                 start=(j == 0), stop=False)
            nc.tensor.matmul(mps_ps[:], lhsT=mps0oT[:, 0, :], rhs=idbf_s[:],
                             start=False, stop=True)
            mps_own = cp.tile([128, DIM], F32, tag="mps_own")
            nc.vector.tensor_copy(mps_own[:], mps_ps[:])

            # l2 normalize rows (free axis)
            nsq = smp.tile([128, 1], F32, tag="nsq")
            nscr = smp.tile([128, DIM], F32, tag="nscr")
            nc.scalar.activation(out=nscr[:], in_=mps_own[:],
                                 func=AFT.Square, accum_out=nsq[:])
            nrm = smp.tile([128, 1], F32, tag="nrm")
            nc.scalar.sqrt(nrm[:], nsq[:])
            nrm2 = smp.tile([128, 1], F32, tag="nrm2")
            nc.vector.tensor_scalar_max(nrm2[:], nrm[:], EPS)
            inv = smp.tile([128, 1], F32, tag="inv")
            nc.vector.reciprocal(inv[:], nrm2[:])
            mps_n = cp.tile([128, DIM], F32, tag="mps_n")  # [a_own, c]
            nc.vector.tensor_scalar_mul(mps_n[:], mps_own[:], inv[:])
            # transpose -> [c, a_own]
            mnt_ps = psB.tile([128, 128], F32, tag="ps")
            nc.tensor.transpose(mnt_ps[:], mps_n[:], idf_s[:])
            mps_nT = cp.tile([128, 128], F32, tag="mps_nT")
            nc.vector.tensor_copy(mps_nT[:], mnt_ps[:])

            # ---- main loop: mpo gather + ssq + diag matmuls ------------
            ssq = cp.tile([128, APC], F32, tag="ssq")   # [c, a_local]
            wT = cp.tile([128, APC], F32, tag="wT")     # [c, a_local]
            tmp_ps = psA.tile([128, N], F32, tag="tmp_ps")  # [c, b] accum

            nch_eff = int(stage[1:]) if stage.startswith("g") else NCH
            for ch in range(nch_eff):
                slab = slabp.tile([128, 1, ACH * N], BF, tag="slab")
                # 4 gather calls of 4096 idxs each (SWDGE ring carveout
                # holds ~1000 descriptors; 258/call leaves pipelining room)
                for q in range(GCALLS):
                    i0 = ch * (ACH * N // 16) + q * (GIDX // 16)
                    nc.gpsimd.dma_gather(
                        out_ap=slab[:, :, q * GIDX:(q + 1) * GIDX],
                        in_ap=e_bond[:],
                        idxs_ap=bidx_s[:, i0:i0 + GIDX // 16],
                        num_idxs=GIDX, num_idxs_reg=GIDX,
                        elem_size=DIM, transpose=True, single_packet=False)

                if stage == "gather":
                    gdump = smp.tile([128, GCALLS], BF, tag="gdump")
                    for q in range(GCALLS):
                        nc.vector.tensor_copy(gdump[:, q:q + 1],
                                              slab[:, 0, q * GIDX:q * GIDX + 1])
                    continue
                # ssq: first DVE_J rows on DVE (square + tree-add),
                # the rest on ACT (Square + accum_out).
                if DVE_J > 0:
                    sq = scrp.tile([128, DVE_J, N], BF, tag="dscr")
                    nc.vector.tensor_mul(
                        sq.rearrange("p j n -> p (j n)"),
                        slab[:, 0, :DVE_J * N], slab[:, 0, :DVE_J * N])
                    # tree-add over b within each row
                    t1 = scrp.tile([128, DVE_J, N // 2], BF, tag="tr1")
                    t2 = scrp.tile([128, DVE_J, N // 4], BF, tag="tr2")
                    nc.vector.tensor_add(t1[:], sq[:, :, :N // 2],
                                         sq[:, :, N // 2:])
                    nc.vector.tensor_add(t2[:], t1[:, :, :N // 4],
                                         t1[:, :, N // 4:])
                    lvls = [t2]
                    w_ = N // 4
                    while w_ > 2:
                        w_ //= 2
                        nxt = scrp.tile([128, DVE_J, w_], BF,
                                        tag=f"tr{w_}")
                        nc.vector.tensor_add(nxt[:], lvls[-1][:, :, :w_],
                                             lvls[-1][:, :, w_:])
                        lvls.append(nxt)
                    # final level -> f32 ssq columns
                    last = lvls[-1]
                    nc.vector.tensor_add(
                        ssq[:, ch * ACH:ch * ACH + DVE_J],
                        last[:, :, 0], last[:, :, 1])
                for j in range(DVE_J, ACH):
                    al = ch * ACH + j
                    scr = scrp.tile([128, N], BF, tag="ascr")
                    nc.scalar.activation(
                        out=scr[:], in_=slab[:, 0, j * N:(j + 1) * N],
                        func=AFT.Square, accum_out=ssq[:, al:al + 1])

                if stage == "ssq":
                    continue
                # w for this chunk: w[c, a] = mps_nT / max(sqrt(ssq), eps)
                c0, c1 = ch * ACH, (ch + 1) * ACH
                st = smp.tile([128, ACH], F32, tag="st")
                nc.scalar.sqrt(st[:], ssq[:, c0:c1])
                st2 = smp.tile([128, ACH], F32, tag="st2")
                nc.vector.tensor_scalar_max(st2[:], st[:], EPS)
                sti = smp.tile([128, ACH], F32, tag="sti")
                nc.vector.reciprocal(sti[:], st2[:])
                nc.vector.tensor_mul(wT[:, c0:c1], mps_nT[:, c0:c1], sti[:])

                # tmp[c, b] += diag(w_a) @ slab_a
                for j in range(ACH):
                    al = ch * ACH + j
                    diag = diagp.tile([128, 128], BF, tag="diag")
                    nc.vector.tensor_scalar_mul(diag[:], idbf_s[:],
                                                wT[:, al:al + 1])
                    nc.tensor.matmul(
                        tmp_ps[:, 0:512], lhsT=diag[:],
                        rhs=slab[:, 0, j * N:j * N + 512],
                        start=(al == 0), stop=(al == nch_eff * ACH - 1),
                        skip_group_check=True)
                    nc.tensor.matmul(
                        tmp_ps[:, 512:1024], lhsT=diag[:],
                        rhs=slab[:, 0, j * N + 512:(j + 1) * N],
                        start=(al == 0), stop=(al == nch_eff * ACH - 1),
                        skip_group_check=True)

            if stage in ("gather", "ssq", "mm"):  # early-exit debug stages
                # debug early-exit: emit a token output and stop
                dbg = smp.tile([MPC, 1], F32, tag="dbg")
                if stage == "gather":
                    nc.vector.tensor_copy(dbg[:], slab[:MPC, 0, 0:1])
                elif stage == "ssq":
                    nc.vector.tensor_copy(dbg[:], ssq[:MPC, 0:1])
                else:
                    tmp_dbg = cp.tile([128, N], F32, tag="tmp_dbg")
                    nc.vector.tensor_copy(tmp_dbg[:], tmp_ps[:])
                    nc.vector.tensor_copy(dbg[:], tmp_dbg[:MPC, 0:1])
                nc.sync.dma_start(out[:], dbg[:])
                nc.compile()
                return nc

            # ---- tmp -> b-major -> ReduceScatter -----------------------
            tmp_s = cp.tile([128, N], F32, tag="tmp_s")
            nc.vector.tensor_copy(tmp_s[:], tmp_ps[:])
            for j in range(NCH):
                tps = psB.tile([128, 128], F32, tag="ps")
                nc.tensor.transpose(tps[:], tmp_s[:, j * 128:(j + 1) * 128],
                                    idf_s[:])
                tts = smp.tile([128, 128], F32, tag="tts")
                nc.vector.tensor_copy(tts[:], tps[:])
                nc.sync.dma_start(cc_in[j * 128:(j + 1) * 128, :], tts[:])

            nc.gpsimd.collective_compute(
                "ReduceScatter", AOT.add,
                replica_groups=[list(range(NCORES))],
                ins=[cc_in[:]], outs=[cc_out[:]])

            # ---- finale: tn + MLP (own 8 molecules) --------------------
            tro = smp.tile([128, DIM], F32, tag="tro")  # [b_own, c]
            nc.sync.dma_start(tro[:], cc_out[:])
            prod = smp.tile([128, DIM], F32, tag="prod")
            nc.vector.tensor_mul(prod[:], tro[:], mps_n[:])
            tn_ps = psB.tile([MPC, DIM], F32, tag="ps")
            nc.tensor.matmul(tn_ps[:], lhsT=sel_s[:], rhs=prod[:],
                             start=True, stop=True)
            tn_s = smp.tile([MPC, DIM], F32, tag="tn_s")
            nc.vector.tensor_copy(tn_s[:], tn_ps[:])
            tnT_ps = psB.tile([128, MPC], F32, tag="ps")
            nc.tensor.transpose(tnT_ps[:], tn_s[:], idf_s[:MPC, :MPC])
            tnT_s = smp.tile([128, MPC], F32, tag="tnT_s")
            nc.vector.tensor_copy(tnT_s[:], tnT_ps[:])

            x0_ps = psB.tile([MPC, DIM], F32, tag="ps")
            nc.tensor.matmul(x0_ps[:], lhsT=tnT_s[:], rhs=w0_s[:],
                             start=True, stop=False)
            nc.tensor.matmul(x0_ps[:], lhsT=onesf_s[:, :MPC], rhs=b0_s[:],
                             start=False, stop=True)
            x0_s = smp.tile([MPC, DIM], F32, tag="x0_s")
            nc.vector.tensor_scalar_max(x0_s[:], x0_ps[:], 0.0)
            x0T_ps = psB.tile([128, MPC], F32, tag="ps")
            nc.tensor.transpose(x0T_ps[:], x0_s[:], idf_s[:MPC, :MPC])
            x0T_s = smp.tile([128, MPC], F32, tag="x0T_s")
            nc.vector.tensor_copy(x0T_s[:], x0T_ps[:])

            x1_ps = psB.tile([MPC, DIM], F32, tag="ps")
            nc.tensor.matmul(x1_ps[:], lhsT=x0T_s[:], rhs=w1_s[:],
                             start=True, stop=False)
            nc.tensor.matmul(x1_ps[:], lhsT=onesf_s[:, :MPC], rhs=b1_s[:],
                             start=False, stop=True)
            x1_s = smp.tile([MPC, DIM], F32, tag="x1_s")
            nc.vector.tensor_scalar_max(x1_s[:], x1_ps[:], 0.0)
            x1T_ps = psB.tile([128, MPC], F32, tag="ps")
            nc.tensor.transpose(x1T_ps[:], x1_s[:], idf_s[:MPC, :MPC])
            x1T_s = smp.tile([128, MPC], F32, tag="x1T_s")
            nc.vector.tensor_copy(x1T_s[:], x1T_ps[:])

            y_ps = psB.tile([MPC, 1], F32, tag="ps")
            nc.tensor.matmul(y_ps[:], lhsT=x1T_s[:], rhs=wp_s[:],
                             start=True, stop=False)
            nc.tensor.matmul(y_ps[:], lhsT=onesf_s[:, :MPC], rhs=bp_s[:, :1],
                             start=False, stop=True)
            y_s = smp.tile([MPC, 1], F32, tag="y_s")
            nc.vector.tensor_copy(y_s[:], y_ps[:])
            nc.sync.dma_start(out[:], y_s[:])

    nc.compile()
    return nc




def build_fast():
    """Fast path: bond_index verified block-diagonal with off-block == 0.

    Gathers only the 2048 in-block E_bond rows per core; the off-block
    contribution (all index 0 -> row T0) is added analytically:
      ssq[a,c]   = ssq_in[a,c] + 1008*T0[c]^2
      tmp[c,b]   = tmp_in[c,b] + T0[c]*(gwsum[c] - wmol[c, mol(b)])
    gwsum is a 512-byte AllReduce of per-core w column sums; everything
    else is core-local, so no ReduceScatter of tmp is needed.
    """
    nc = bacc.Bacc("TRN2", target_bir_lowering=False, debug=False,
                   num_devices=NCORES)

    e_bond = nc.declare_dram_parameter("e_bond", [N_BOND, DIM], BF, False)
    e_fp = nc.declare_dram_parameter("e_fp", [N_FP, DIM], BF, False)
    fpi_all = nc.declare_dram_parameter("fpi_all", [128, N // 16], I16, False)
    fpi_own = nc.declare_dram_parameter("fpi_own", [128, APC // 16], I16, False)
    ibx = nc.declare_dram_parameter("ibx", [128, APC * NA // 16], I16, False)
    at = nc.declare_dram_parameter("at", [N, APC], BF, False)
    wfp = nc.declare_dram_parameter("wfp", [DIM, DIM], BF, False)
    bfp = nc.declare_dram_parameter("bfp", [1, DIM], BF, False)
    w0 = nc.declare_dram_parameter("w0", [DIM, DIM], F32, False)
    b0 = nc.declare_dram_parameter("b0", [1, DIM], F32, False)
    w1 = nc.declare_dram_parameter("w1", [DIM, DIM], F32, False)
    b1 = nc.declare_dram_parameter("b1", [1, DIM], F32, False)
    wp = nc.declare_dram_parameter("wp", [DIM, 1], F32, False)
    bp = nc.declare_dram_parameter("bp", [1, 1], F32, False)
    idbf = nc.declare_dram_parameter("idbf", [128, 128], BF, False)
    idf = nc.declare_dram_parameter("idf", [128, 128], F32, False)
    onesbf = nc.declare_dram_parameter("onesbf", [1, 128], BF, False)
    onesf = nc.declare_dram_parameter("onesf", [1, 128], F32, False)
    t0col = nc.declare_dram_parameter("t0col", [128, 1], F32, False)
    out = nc.declare_dram_parameter("out", [MPC, 1], F32, True)

    cc_in = nc.dram_tensor("cc_in", [128, 1], F32)
    cc_out = nc.dram_tensor("cc_out", [128, 1], F32, addr_space="Shared")
    dummy_in = nc.dram_tensor("dummy_in", [1, 1], F32)
    dummy_out = nc.dram_tensor("dummy_out", [NCORES, 1], F32,
                               addr_space="Shared")

    NIB = APC * NA  # 2048 in-block indices per core

    with tile.TileContext(nc) as tc:
        with (
            tc.tile_pool(name="const", bufs=1) as cp,
            tc.tile_pool(name="small", bufs=2) as smp,
            tc.tile_pool(name="psB", bufs=3, space="PSUM") as psB,
        ):
            # Fire a dummy collective first: NRT's first-collective init
            # barrier (~40us, absorbs inter-core dispatch skew) then runs
            # concurrently with the local compute below, so the real
            # AllReduce later is cheap.
            nc.gpsimd.collective_compute(
                "AllGather", AOT.bypass,
                replica_groups=[list(range(NCORES))],
                ins=[dummy_in[:]], outs=[dummy_out[:]])

            # ---- constants -------------------------------------------
            wfp_s = cp.tile([DIM, DIM], BF, tag="wfp_s")
            nc.sync.dma_start(wfp_s[:], wfp[:])
            idbf_s = cp.tile([128, 128], BF, tag="idbf_s")
            nc.sync.dma_start(idbf_s[:], idbf[:])
            idf_s = cp.tile([128, 128], F32, tag="idf_s")
            nc.sync.dma_start(idf_s[:], idf[:])
            onesbf_s = cp.tile([1, 128], BF, tag="onesbf_s")
            nc.sync.dma_start(onesbf_s[:], onesbf[:])
            onesf_s = cp.tile([1, 128], F32, tag="onesf_s")
            nc.sync.dma_start(onesf_s[:], onesf[:])
            bfp_s = cp.tile([1, DIM], BF, tag="bfp_s")
            nc.sync.dma_start(bfp_s[:], bfp[:])
            w0_s = cp.tile([DIM, DIM], F32, tag="w0_s")
            nc.sync.dma_start(w0_s[:], w0[:])
            b0_s = cp.tile([1, DIM], F32, tag="b0_s")
            nc.sync.dma_start(b0_s[:], b0[:])
            w1_s = cp.tile([DIM, DIM], F32, tag="w1_s")
            nc.sync.dma_start(w1_s[:], w1[:])
            b1_s = cp.tile([1, DIM], F32, tag="b1_s")
            nc.sync.dma_start(b1_s[:], b1[:])
            wp_s = cp.tile([DIM, 1], F32, tag="wp_s")
            nc.sync.dma_start(wp_s[:], wp[:])
            bp_s = cp.tile([1, 1], F32, tag="bp_s")
            nc.sync.dma_start(bp_s[:], bp[:])
            fpi_all_s = cp.tile([128, N // 16], I16, tag="fpi_all_s")
            nc.sync.dma_start(fpi_all_s[:], fpi_all[:])
            fpi_own_s = cp.tile([128, APC // 16], I16, tag="fpi_own_s")
            nc.sync.dma_start(fpi_own_s[:], fpi_own[:])
            ibx_s = cp.tile([128, NIB // 16], I16, tag="ibx_s")
            nc.sync.dma_start(ibx_s[:], ibx[:])
            at_s = cp.tile([128, NCH, 128], BF, tag="at_s")
            for j in range(NCH):
                nc.sync.dma_start(at_s[:, j, :], at[j * 128:(j + 1) * 128, :])
            t0_s = cp.tile([128, 1], F32, tag="t0_s")
            nc.sync.dma_start(t0_s[:], t0col[:])

            # ---- gathers (start immediately, overlap mps stage) ------
            inb = cp.tile([128, 1, NIB], BF, tag="inb")  # [c, (m,a,n)]
            nc.gpsimd.dma_gather(
                out_ap=inb[:], in_ap=e_bond[:], idxs_ap=ibx_s[:],
                num_idxs=NIB, num_idxs_reg=NIB, elem_size=DIM,
                transpose=True, single_packet=False)
            mps0T = cp.tile([128, 1, N], BF, tag="mps0T")
            nc.gpsimd.dma_gather(
                out_ap=mps0T[:], in_ap=e_fp[:], idxs_ap=fpi_all_s[:],
                num_idxs=N, num_idxs_reg=N, elem_size=DIM, transpose=True,
                single_packet=False)
            mps0oT = cp.tile([128, 1, APC], BF, tag="mps0oT")
            nc.gpsimd.dma_gather(
                out_ap=mps0oT[:], in_ap=e_fp[:], idxs_ap=fpi_own_s[:],
                num_idxs=APC, num_idxs_reg=APC, elem_size=DIM,
                transpose=True, single_packet=False)

            # ---- MPS stage (identical to slow path) ------------------
            contri_s = cp.tile([128, NCH, DIM], BF, tag="contri_s")
            for j in range(NCH):
                cps = psB.tile([128, DIM], F32, tag="ps")
                nc.tensor.matmul(cps[:], lhsT=mps0T[:, 0, j * 128:(j + 1) * 128],
                                 rhs=wfp_s[:], start=True, stop=False)
                nc.tensor.matmul(cps[:], lhsT=onesbf_s[:], rhs=bfp_s[:],
                                 start=False, stop=True)
                nc.vector.tensor_scalar_max(contri_s[:, j, :], cps[:], 0.0)

            mps_ps = psB.tile([128, DIM], F32, tag="ps")
            for j in range(NCH):
                nc.tensor.matmul(mps_ps[:], lhsT=at_s[:, j, :],
                                 rhs=contri_s[:, j, :],
                                 start=(j == 0), stop=False)
            nc.tensor.matmul(mps_ps[:], lhsT=mps0oT[:, 0, :], rhs=idbf_s[:],
                             start=False, stop=True)
            mps_own = cp.tile([128, DIM], F32, tag="mps_own")
            nc.vector.tensor_copy(mps_own[:], mps_ps[:])

            nsq = smp.tile([128, 1], F32, tag="nsq")
            nscr = smp.tile([128, DIM], F32, tag="nscr")
            nc.scalar.activation(out=nscr[:], in_=mps_own[:],
                                 func=AFT.Square, accum_out=nsq[:])
            nrm = smp.tile([128, 1], F32, tag="nrm")
            nc.scalar.sqrt(nrm[:], nsq[:])
            nrm2 = smp.tile([128, 1], F32, tag="nrm2")
            nc.vector.tensor_scalar_max(nrm2[:], nrm[:], EPS)
            inv = smp.tile([128, 1], F32, tag="inv")
            nc.vector.reciprocal(inv[:], nrm2[:])
            mps_n = cp.tile([128, DIM], F32, tag="mps_n")  # [a_own, c]
            nc.vector.tensor_scalar_mul(mps_n[:], mps_own[:], inv[:])
            mnt_ps = psB.tile([128, 128], F32, tag="ps")
            nc.tensor.transpose(mnt_ps[:], mps_n[:], idf_s[:])
            mps_nT = cp.tile([128, 128], F32, tag="mps_nT")  # [c, a_own]
            nc.vector.tensor_copy(mps_nT[:], mnt_ps[:])

            # ---- ssq from in-block slab + analytic T0 part -----------
            sq = cp.tile([128, NIB], BF, tag="sq")
            nc.vector.tensor_mul(sq[:], inb[:, 0, :], inb[:, 0, :])
            ssq_in = cp.tile([128, APC], F32, tag="ssq_in")  # [c, (m,a)]
            nc.vector.reduce_sum(
                ssq_in.rearrange("p (g o) -> p g o", o=1),
                sq.rearrange("p (g n) -> p g n", n=NA),
                axis=mybir.AxisListType.X)
            # t0sq1008 = 1008 * T0^2
            t0sq = smp.tile([128, 1], F32, tag="t0sq")
            nc.scalar.activation(out=t0sq[:], in_=t0_s[:], func=AFT.Square)
            t0sqk = smp.tile([128, 1], F32, tag="t0sqk")
            nc.vector.tensor_scalar_mul(t0sqk[:], t0sq[:], float(N - NA))
            ssq = cp.tile([128, APC], F32, tag="ssq")
            nc.vector.tensor_scalar(ssq[:], ssq_in[:], t0sqk[:], None,
                                    op0=AOT.add)

            # ---- w = mps_nT / max(sqrt(ssq), eps);  [c, a_local] ------
            st = smp.tile([128, APC], F32, tag="st")
            nc.scalar.sqrt(st[:], ssq[:])
            st2 = smp.tile([128, APC], F32, tag="st2")
            nc.vector.tensor_scalar_max(st2[:], st[:], EPS)
            sti = smp.tile([128, APC], F32, tag="sti")
            nc.vector.reciprocal(sti[:], st2[:])
            wT = cp.tile([128, APC], F32, tag="wT")
            nc.vector.tensor_mul(wT[:], mps_nT[:], sti[:])

            # ---- gwsum AllReduce (512 B) ------------------------------
            wsum = smp.tile([128, 1], F32, tag="wsum")
            nc.vector.reduce_sum(wsum[:], wT[:], axis=mybir.AxisListType.X)
            nc.sync.dma_start(cc_in[:], wsum[:])
            nc.gpsimd.collective_compute(
                "AllReduce", AOT.add, replica_groups=[list(range(NCORES))],
                ins=[cc_in[:]], outs=[cc_out[:]])
            gw = smp.tile([128, 1], F32, tag="gw")
            nc.sync.dma_start(gw[:], cc_out[:])

            # wmol[c, m] = sum of w over mol m's 16 atoms
            wmol = smp.tile([128, MPC], F32, tag="wmol")
            nc.vector.reduce_sum(
                wmol.rearrange("p (g o) -> p g o", o=1),
                wT.rearrange("p (g n) -> p g n", n=NA),
                axis=mybir.AxisListType.X)
            # tadd[c, m] = T0[c] * (gwsum[c] - wmol[c, m])
            d1 = smp.tile([128, MPC], F32, tag="d1")
            nc.vector.tensor_scalar(d1[:], wmol[:], gw[:], None,
                                    op0=AOT.subtract)  # wmol - gw
            nt0 = smp.tile([128, 1], F32, tag="nt0")
            nc.vector.tensor_scalar_mul(nt0[:], t0_s[:], -1.0)
            tadd = smp.tile([128, MPC], F32, tag="tadd")
            nc.vector.tensor_scalar(tadd[:], d1[:], nt0[:], None,
                                    op0=AOT.mult)  # (gw-wmol)*T0

            # ---- tmp_in[c, b_local] = sum_a w[a,c]*inb[c,(m,a,n)] -----
            prod = cp.tile([128, NIB], F32, tag="prod")
            for m in range(MPC):
                for a_l in range(NA):
                    al = m * NA + a_l
                    csl = slice(al * NA, (al + 1) * NA)
                    nc.vector.tensor_scalar_mul(prod[:, csl],
                                                inb[:, 0, csl],
                                                wT[:, al:al + 1])
            tmp_in = cp.tile([128, APC], F32, tag="tmp_in")  # [c,(m,n)]
            nc.vector.reduce_sum(
                tmp_in.rearrange("p (m n o) -> p m n o", n=NA, o=1),
                prod.rearrange("p (m a n) -> p m n a", a=NA, n=NA),
                axis=mybir.AxisListType.X)

            # tmp_own[c, (m,n)] = tmp_in + tadd[c, m]
            tmp_own = cp.tile([128, APC], F32, tag="tmp_own")
            for m in range(MPC):
                msl = slice(m * NA, (m + 1) * NA)
                nc.vector.tensor_scalar(tmp_own[:, msl], tmp_in[:, msl],
                                        tadd[:, m:m + 1], None, op0=AOT.add)

            # ---- finale in [c, b] layout ------------------------------
            fprod = smp.tile([128, APC], F32, tag="fprod")
            nc.vector.tensor_mul(fprod[:], tmp_own[:], mps_nT[:])
            tnT = smp.tile([128, MPC], F32, tag="tnT")  # [d, m]
            nc.vector.reduce_sum(
                tnT.rearrange("p (g o) -> p g o", o=1),
                fprod.rearrange("p (g n) -> p g n", n=NA),
                axis=mybir.AxisListType.X)

            x0_ps = psB.tile([MPC, DIM], F32, tag="ps")
            nc.tensor.matmul(x0_ps[:], lhsT=tnT[:], rhs=w0_s[:],
                             start=True, stop=False)
            nc.tensor.matmul(x0_ps[:], lhsT=onesf_s[:, :MPC], rhs=b0_s[:],
                             start=False, stop=True)
            x0_s = smp.tile([MPC, DIM], F32, tag="x0_s")
            nc.vector.tensor_scalar_max(x0_s[:], x0_ps[:], 0.0)
            x0T_ps = psB.tile([128, MPC], F32, tag="ps")
            nc.tensor.transpose(x0T_ps[:], x0_s[:], idf_s[:MPC, :MPC])
            x0T_s = smp.tile([128, MPC], F32, tag="x0T_s")
            nc.vector.tensor_copy(x0T_s[:], x0T_ps[:])

            x1_ps = psB.tile([MPC, DIM], F32, tag="ps")
            nc.tensor.matmul(x1_ps[:], lhsT=x0T_s[:], rhs=w1_s[:],
                             start=True, stop=False)
            nc.tensor.matmul(x1_ps[:], lhsT=onesf_s[:, :MPC], rhs=b1_s[:],
                             start=False, stop=True)
            x1_s = smp.tile([MPC, DIM], F32, tag="x1_s")
            nc.vector.tensor_scalar_max(x1_s[:], x1_ps[:], 0.0)
            x1T_ps = psB.tile([128, MPC], F32, tag="ps")
            nc.tensor.transpose(x1T_ps[:], x1_s[:], idf_s[:MPC, :MPC])
            x1T_s = smp.tile([128, MPC], F32, tag="x1T_s")
            nc.vector.tensor_copy(x1T_s[:], x1T_ps[:])

            y_ps = psB.tile([MPC, 1], F32, tag="ps")
            nc.tensor.matmul(y_ps[:], lhsT=x1T_s[:], rhs=wp_s[:],
                             start=True, stop=False)
            nc.tensor.matmul(y_ps[:], lhsT=onesf_s[:, :MPC], rhs=bp_s[:, :1],
                             start=False, stop=True)
            y_s = smp.tile([MPC, 1], F32, tag="y_s")
            nc.vector.tensor_copy(y_s[:], y_ps[:])
            nc.sync.dma_start(out[:], y_s[:])

    nc.compile()
    return nc


def make_in_maps_fast(fingerprints, adjacency, bond_index, E_fp, E_bond,
                      W_fp, b_fp, W_out0, b_out0, W_out1, b_out1, W_prop,
                      b_prop):
    e_bond_bf = np.ascontiguousarray(E_bond.astype(BF16))
    e_fp_bf = np.ascontiguousarray(E_fp.astype(BF16))
    wfp_bf = np.ascontiguousarray(W_fp.astype(BF16))
    bfp_bf = b_fp.astype(BF16).reshape(1, DIM)
    w0_f = np.ascontiguousarray(W_out0.astype(np.float32))
    b0_f = b_out0.astype(np.float32).reshape(1, DIM)
    w1_f = np.ascontiguousarray(W_out1.astype(np.float32))
    b1_f = b_out1.astype(np.float32).reshape(1, DIM)
    wp_f = np.ascontiguousarray(W_prop.astype(np.float32))
    bp_f = b_prop.astype(np.float32).reshape(1, 1)
    idbf = np.eye(128, dtype=BF16)
    idf = np.eye(128, dtype=np.float32)
    onesbf = np.ones((1, 128), dtype=BF16)
    onesf = np.ones((1, 128), dtype=np.float32)
    # T0 as the bf16-rounded row (matches the gathered slab precision)
    t0 = E_bond[0].astype(BF16).astype(np.float32).reshape(128, 1)
    fpi_all = _wrap_idx16(fingerprints)

    in_maps = []
    for k in range(NCORES):
        rows = slice(k * APC, (k + 1) * APC)
        at_k = np.ascontiguousarray(adjacency[rows, :].T.astype(BF16))
        fpi_own = _wrap_idx16(fingerprints[rows])
        # in-block indices, order (m, a_local, n)
        ib = np.empty(APC * NA, dtype=np.int16)
        pos = 0
        for m in range(MPC):
            gm = k * MPC + m
            blk = bond_index[gm * NA:(gm + 1) * NA, gm * NA:(gm + 1) * NA]
            ib[pos:pos + NA * NA] = blk.astype(np.int16).reshape(-1)
            pos += NA * NA
        in_maps.append({
            "e_bond": e_bond_bf, "e_fp": e_fp_bf,
            "fpi_all": fpi_all, "fpi_own": fpi_own,
            "ibx": _wrap_idx16(ib),
            "at": at_k, "wfp": wfp_bf, "bfp": bfp_bf,
            "w0": w0_f, "b0": b0_f, "w1": w1_f, "b1": b1_f,
            "wp": wp_f, "bp": bp_f,
            "idbf": idbf, "idf": idf, "onesbf": onesbf, "onesf": onesf,
            "t0col": t0,
        })
    return in_maps


def _offblock_is_zero(bond_index):
    mol = np.arange(N) // NA
    block = mol[:, None] == mol[None, :]
    return bool(np.all(np.asarray(bond_index)[~block] == 0))


_NC_CACHE = {}


def _get_nc(fast):
    key = "fast" if fast else "full"
    if key not in _NC_CACHE:
        _NC_CACHE[key] = build_fast() if fast else build_nc()
    return _NC_CACHE[key]


def make_in_maps(fingerprints, adjacency, bond_index, E_fp, E_bond, W_fp,
                 b_fp, W_out0, b_out0, W_out1, b_out1, W_prop, b_prop):
    e_bond_bf = np.ascontiguousarray(E_bond.astype(BF16))
    e_fp_bf = np.ascontiguousarray(E_fp.astype(BF16))
    wfp_bf = np.ascontiguousarray(W_fp.astype(BF16))
    bfp_bf = b_fp.astype(BF16).reshape(1, DIM)
    w0_f = np.ascontiguousarray(W_out0.astype(np.float32))
    b0_f = b_out0.astype(np.float32).reshape(1, DIM)
    w1_f = np.ascontiguousarray(W_out1.astype(np.float32))
    b1_f = b_out1.astype(np.float32).reshape(1, DIM)
    wp_f = np.ascontiguousarray(W_prop.astype(np.float32))
    bp_f = b_prop.astype(np.float32).reshape(1, 1)
    idbf = np.eye(128, dtype=BF16)
    idf = np.eye(128, dtype=np.float32)
    onesbf = np.ones((1, 128), dtype=BF16)
    onesf = np.ones((1, 128), dtype=np.float32)
    # molecule-sum selector: sel[b, m] = 1 if b // 16 == m
    sel = np.zeros((128, MPC), dtype=np.float32)
    for m in range(MPC):
        sel[m * NA:(m + 1) * NA, m] = 1.0

    fpi_all = _wrap_idx16(fingerprints)

    in_maps = []
    for k in range(NCORES):
        rows = slice(k * APC, (k + 1) * APC)
        # bond idx: wrapped per gather call (each call wraps its own idxs)
        flat = bond_index[rows, :].astype(np.int16).reshape(
            NCH * GCALLS, GIDX)
        bidx = np.concatenate(
            [_wrap_idx16(flat[c]) for c in range(NCH * GCALLS)],
            axis=1)  # [128, 8192]
        at_k = np.ascontiguousarray(adjacency[rows, :].T.astype(BF16))
        fpi_own = _wrap_idx16(fingerprints[rows])
        in_maps.append({
            "e_bond": e_bond_bf, "e_fp": e_fp_bf,
            "fpi_all": fpi_all, "fpi_own": fpi_own, "bidx": bidx,
            "at": at_k, "wfp": wfp_bf, "bfp": bfp_bf,
            "w0": w0_f, "b0": b0_f, "w1": w1_f, "b1": b1_f,
            "wp": wp_f, "bp": bp_f,
            "idbf": idbf, "idf": idf, "onesbf": onesbf, "onesf": onesf,
            "sel": sel,
        })
    return in_maps


def run(inputs, trace=False, **kw):
    fast = _offblock_is_zero(inputs["bond_index"])
    nc = _get_nc(fast)
    in_maps = make_in_maps_fast(**inputs) if fast else make_in_maps(**inputs)
    res = run_bass_kernel_spmd(nc, in_maps, core_ids=list(range(NCORES)),
                               trace=trace, **kw)
    out = np.concatenate([res.results[k]["out"] for k in range(NCORES)],
                         axis=0).astype(np.float32)
    return out, res


def kernel(**inputs):
    out, _ = run(inputs, trace=False)
    return out
